# revision 1
# baseline (speedup 1.0000x reference)
"""Self-contained Trainium2 Bass kernel for the 2-layer dual-graph GCN
(nn_GCN0100). Accepts FULL inputs, returns FULL output.

Strategy: node-sharded across 8 NeuronCores, 3 SPMD-style launches:
  run1: h = x @ W1 per shard (fp16 tables)
  run2: layer-1 gather/segment-sum over both graphs (dma_gather + one-hot
        matmul reduction into PSUM), ReLU+bias, h2 = R1 @ W2
  run3: layer-2 gather/segment-sum, logits, log_softmax
Host assembles the full fp16 feature tables between launches (the "halo
exchange") and does index-only graph partitioning; all FLOPs run on device.
"""
import threading
import time
import numpy as np
import jax
import concourse.bass as bass
import concourse.mybir as mybir
import concourse.tile as tile
from concourse import bacc
from concourse.bass2jax import _bass_exec_p, partition_id_tensor, install_neuronx_cc_hook




P = 128
SH = 12800          # shard size (102400 / 8)
NPAD = 102400       # padded node count
CH = 25600          # gather chunk rows (fits int16)
BANK = 512          # PSUM bank slots
STILE = 4096        # S stream SBUF tile free size (fp16 elems per partition)
ITILE = 2048        # idx stream SBUF tile free size (int16 elems per partition)
NIDX_MAX = 6144     # max indices per dma_gather call


def degrees_dinv(edge_index, n=100000):
    deg = np.bincount(np.asarray(edge_index[1]), minlength=n).astype(np.float64) + 1.0
    return (1.0 / np.sqrt(deg)).astype(np.float32)


def build_shard_plan(edge_index, dinv, core):
    """Returns plan dict for one (graph, core) pair."""
    n0 = core * SH
    n1 = n0 + SH
    row = np.asarray(edge_index[0]).astype(np.int64)
    col = np.asarray(edge_index[1]).astype(np.int64)
    m = (col >= n0) & (col < n1)
    row, col = row[m], col[m]
    # self loops for real nodes in shard (nodes >= 100000 are padding)
    selfn = np.arange(n0, min(n1, 100000), dtype=np.int64)
    row = np.concatenate([row, selfn])
    col = np.concatenate([col, selfn])
    norm = (dinv[row] * dinv[col]).astype(np.float32)

    slot = (col - n0).astype(np.int32)
    bank = slot >> 9
    chunk = (row // CH).astype(np.int32)
    lidx = (row % CH).astype(np.int32)

    order = np.lexsort((slot, chunk, bank))
    slot, bank, chunk, lidx, norm = (
        slot[order], bank[order], chunk[order], lidx[order], norm[order]
    )

    nbanks = (SH + BANK - 1) // BANK
    # cell boundaries
    cells = []  # (bank, chunk, idx_arr int32, slot_arr, norm_arr) padded to 128-mult
    key = bank.astype(np.int64) * 8 + chunk
    uniq, starts = np.unique(key, return_index=True)
    starts = np.sort(starts)
    bounds = list(starts) + [len(key)]
    for s, e in zip(bounds[:-1], bounds[1:]):
        b, c = int(bank[s]), int(chunk[s])
        li, sl, nm = lidx[s:e], slot[s:e], norm[s:e]
        pad = (-len(li)) % P
        if pad:
            li = np.concatenate([li, np.full(pad, li[-1], np.int32)])
            sl = np.concatenate([sl, np.full(pad, sl[-1], np.int32)])
            nm = np.concatenate([nm, np.zeros(pad, np.float32)])
        cells.append((b, c, li, sl, nm))

    # gather calls: one call per cell (keeps at most one call live per
    # (bank, chunk) step of the emit loop -> small tile-pool liveness)
    calls = []
    call_of_cell = {}
    for ci, cell in enumerate(cells):
        call_of_cell[ci] = (len(calls), 0)
        calls.append({"chunk": cell[1], "n": len(cell[2]), "idx": cell[2]})

    # windows: per cell, chop into 128-edge windows; emit metadata + S blocks
    windows = []  # (graph-level) dicts: call_id, wslot, bank, smin, B, s_off
    s_blocks = []
    s_off = 0
    for ci, (b, c, li, sl, nm) in enumerate(cells):
        call_id, wbase = call_of_cell[ci]
        nw = len(li) // P
        for w in range(nw):
            ssl = sl[w * P:(w + 1) * P]
            snm = nm[w * P:(w + 1) * P]
            smin = int(ssl.min())
            smax = int(ssl.max())
            B = smax - smin + 1
            S = np.zeros((P, B), np.float16)
            S[np.arange(P), ssl - smin] = snm.astype(np.float16)
            windows.append({
                "call": call_id, "wslot": wbase + w, "bank": b,
                "smin": smin - b * BANK, "B": B, "s_off": s_off,
            })
            s_blocks.append(S)
            s_off += B
    return {
        "cells": cells, "calls": calls, "windows": windows,
        "s_blocks": s_blocks, "nbanks": nbanks,
    }


def pack_streams(plan):
    """Build upload arrays: S stream [128, STOT] fp16 (tile-aligned),
    idx stream [128, ITOT] int16 (call slices tile-aligned, wrapped+replicated),
    and rewrite window/call metadata with tile-local offsets."""
    # S stream
    s_tiles_used = 0
    cur = 0
    offs = []
    for w, S in zip(plan["windows"], plan["s_blocks"]):
        B = w["B"]
        if cur + B > STILE:
            s_tiles_used += 1
            cur = 0
        offs.append((s_tiles_used, cur))
        cur += B
    n_stiles = s_tiles_used + 1
    s_arr = np.zeros((P, n_stiles * STILE), np.float16)
    for (tile_i, off), w, S in zip(offs, plan["windows"], plan["s_blocks"]):
        w["s_tile"] = tile_i
        w["s_col"] = off
        s_arr[:, tile_i * STILE + off: tile_i * STILE + off + w["B"]] = S

    # idx stream: per call, wrapped [16, n/16] replicated to 128 partitions
    i_tiles_used = 0
    cur = 0
    for call in calls_list(plan):
        ncols = call["n"] // 16
        if cur + ncols > ITILE:
            i_tiles_used += 1
            cur = 0
        call["i_tile"] = i_tiles_used
        call["i_col"] = cur
        cur += ncols
    n_itiles = i_tiles_used + 1
    i_arr = np.zeros((P, n_itiles * ITILE), np.int16)
    for call in calls_list(plan):
        idx = call["idx"].astype(np.int16)
        wrapped = idx.reshape(-1, 16).T  # [16, n/16]
        rep = np.tile(wrapped, (8, 1))   # [128, n/16]
        c0 = call["i_tile"] * ITILE + call["i_col"]
        i_arr[:, c0: c0 + wrapped.shape[1]] = rep
    plan["s_arr"] = s_arr
    plan["i_arr"] = i_arr
    plan["n_stiles"] = n_stiles
    plan["n_itiles"] = n_itiles
    return plan


def calls_list(plan):
    return plan["calls"]


# ---------------- numpy emulation of the device algorithm ----------------

def emu_aggregate(plan, table, out_feat):
    """Emulate gathers + window matmuls. table: [NPAD, >=out_feat] fp16.
    Returns aggT [out_feat, SH] float32 (transposed orientation)."""
    nb = plan["nbanks"]
    agg = np.zeros((out_feat, nb * BANK), np.float32)
    gathered = {}
    for cid, call in enumerate(plan["calls"]):
        c = call["chunk"]
        rows = table[c * CH + call["idx"].astype(np.int64)]  # [n, F]
        gathered[cid] = rows
    for w in plan["windows"]:
        g = gathered[w["call"]][w["wslot"] * P:(w["wslot"] + 1) * P, :out_feat]
        S = plan["s_arr"][:, w["s_tile"] * STILE + w["s_col"]:
                          w["s_tile"] * STILE + w["s_col"] + w["B"]]
        # matmul: out[feat, slot] += g[e, feat].T @ S[e, slot]
        contrib = g.astype(np.float32).T @ S.astype(np.float32)
        b0 = w["bank"] * BANK + w["smin"]
        agg[:, b0: b0 + w["B"]] += contrib
    return agg[:, :SH]




F16 = mybir.dt.float16
F32 = mybir.dt.float32
I16 = mybir.dt.int16
NBANK = SH // BANK          # 25
KX = 512 // P               # 4 k-chunks for x@W1


def build_run1():
    """h = x @ W1 for one shard (identical program for all cores).
    Inputs: xT [512, SH] f16, w1 [512, 128] f16. Output: h [SH, 128] f16."""
    nc = bacc.Bacc(None, target_bir_lowering=False)
    xT = nc.dram_tensor("xT", [512, SH], F16, kind="ExternalInput")
    w1 = nc.dram_tensor("w1", [512, 128], F16, kind="ExternalInput")
    h = nc.dram_tensor("h", [SH, 128], F16, kind="ExternalOutput")
    with tile.TileContext(nc) as tc:
        with (
            tc.tile_pool(name="const", bufs=1) as cp,
            tc.tile_pool(name="sb", bufs=3) as sb,
            tc.tile_pool(name="ev", bufs=3) as ev,
            tc.tile_pool(name="ps", bufs=2, space="PSUM") as ps,
        ):
            w1t = cp.tile([128, KX, 128], F16)
            for kc in range(KX):
                nc.sync.dma_start(out=w1t[:, kc, :], in_=w1[kc * 128:(kc + 1) * 128, :])
            for t in range(SH // 512):
                xt = sb.tile([128, KX, 512], F16, tag="xt")
                for kc in range(KX):
                    nc.sync.dma_start(
                        out=xt[:, kc, :],
                        in_=xT[kc * 128:(kc + 1) * 128, t * 512:(t + 1) * 512])
                for s in range(4):
                    pt = ps.tile([128, 128], F32, tag="h")
                    for kc in range(KX):
                        nc.tensor.matmul(
                            out=pt[:], lhsT=xt[:, kc, s * 128:(s + 1) * 128],
                            rhs=w1t[:, kc, :], start=(kc == 0), stop=(kc == KX - 1))
                    he = ev.tile([128, 128], F16, tag="he")
                    nc.vector.tensor_copy(he[:], pt[:])
                    nc.sync.dma_start(
                        out=h[(t * 4 + s) * 128:(t * 4 + s + 1) * 128, :], in_=he[:])
    nc.compile()
    return nc


class AggEmitter:
    """Emits gather calls + window matmuls for one graph, bank at a time."""

    def __init__(self, nc, sb, ps, plan, table, nfeat, tag):
        self.nc, self.sb, self.ps = nc, sb, ps
        self.plan, self.table, self.nfeat, self.tag = plan, table, nfeat, tag
        self.call_tiles = {}
        self.s_tiles = {}
        # windows grouped by bank (plan windows are in (bank, chunk) order)
        self.by_bank = {}
        for w in plan["windows"]:
            self.by_bank.setdefault(w["bank"], []).append(w)

    def _call_tile(self, cid):
        if cid not in self.call_tiles:
            call = self.plan["calls"][cid]
            n = call["n"]
            gt = self.sb.tile([128, n // 128, 128], F16, tag=self.tag + "g")
            it = self.sb.tile([128, n // 16], I16, tag=self.tag + "i")
            c0 = call["i_tile"] * ITILE + call["i_col"]
            self.nc.sync.dma_start(out=it[:], in_=self.plan["dram_i"][:, c0:c0 + n // 16])
            c = call["chunk"]
            self.nc.gpsimd.dma_gather(
                gt[:], self.table[c * CH:(c + 1) * CH, :], it[:], n, n, 128,
                single_packet=False)
            if len(self.call_tiles) > 6:
                for k in sorted(self.call_tiles)[:-5]:
                    del self.call_tiles[k]
            self.call_tiles[cid] = gt
        return self.call_tiles[cid]

    def _s_tile(self, ti):
        if ti not in self.s_tiles:
            st = self.sb.tile([128, STILE], F16, tag=self.tag + "s")
            self.nc.sync.dma_start(
                out=st[:], in_=self.plan["dram_s"][:, ti * STILE:(ti + 1) * STILE])
            if len(self.s_tiles) > 2:
                for k in sorted(self.s_tiles)[:-1]:
                    del self.s_tiles[k]
            self.s_tiles[ti] = st
        return self.s_tiles[ti]

    def emit_bank(self, b):
        """Returns the accumulated PSUM tile [nfeat(pad 128), BANK] for bank b."""
        nc = self.nc
        pt = self.ps.tile([128, BANK], F32, tag=self.tag + "p")
        nc.vector.memset(pt[:self.nfeat, :], 0.0)
        for w in self.by_bank.get(b, []):
            gt = self._call_tile(w["call"])
            st = self._s_tile(w["s_tile"])
            nc.tensor.matmul(
                out=pt[:self.nfeat, w["smin"]:w["smin"] + w["B"]],
                lhsT=gt[:, w["wslot"], :self.nfeat],
                rhs=st[:, w["s_col"]:w["s_col"] + w["B"]],
                start=False, stop=True, skip_group_check=True)
        return pt


def build_run2(plan_s, plan_k):
    """L1 aggregation (both graphs) + R1 + h2 = R1 @ W2 for one core."""
    nc = bacc.Bacc(None, target_bir_lowering=False)
    tb = nc.dram_tensor("tb", [NPAD, 128], F16, kind="ExternalInput")
    sa = nc.dram_tensor("sa", [128, plan_s["n_stiles"] * STILE], F16, kind="ExternalInput")
    ia = nc.dram_tensor("ia", [128, plan_s["n_itiles"] * ITILE], I16, kind="ExternalInput")
    sk = nc.dram_tensor("sk", [128, plan_k["n_stiles"] * STILE], F16, kind="ExternalInput")
    ik = nc.dram_tensor("ik", [128, plan_k["n_itiles"] * ITILE], I16, kind="ExternalInput")
    w2 = nc.dram_tensor("w2", [256, 40], F16, kind="ExternalInput")
    b1v = nc.dram_tensor("b1v", [128, 1], F32, kind="ExternalInput")
    h2 = nc.dram_tensor("h2", [SH, 128], F16, kind="ExternalOutput")
    plan_s["dram_s"], plan_s["dram_i"] = sa, ia
    plan_k["dram_s"], plan_k["dram_i"] = sk, ik
    with tile.TileContext(nc) as tc:
        with (
            tc.tile_pool(name="const", bufs=1) as cp,
            tc.tile_pool(name="sb", bufs=3) as sb,
            tc.tile_pool(name="r1", bufs=2) as r1p,
            tc.tile_pool(name="ev", bufs=3) as ev,
            tc.tile_pool(name="ps", bufs=2, space="PSUM") as ps,
            tc.tile_pool(name="ps2", bufs=2, space="PSUM") as ps2,
        ):
            w2t = cp.tile([128, 2, 40], F16)
            for kc in range(2):
                nc.sync.dma_start(out=w2t[:, kc, :], in_=w2[kc * 128:(kc + 1) * 128, :])
            b1t = cp.tile([128, 1], F32)
            nc.sync.dma_start(out=b1t[:], in_=b1v[:])

            es = AggEmitter(nc, sb, ps, plan_s, tb, 128, "s")
            ek = AggEmitter(nc, sb, ps, plan_k, tb, 128, "k")
            for b in range(NBANK):
                pa = es.emit_bank(b)
                pb = ek.emit_bank(b)
                r1a = r1p.tile([128, BANK], F16, tag="r1a")
                r1b = r1p.tile([128, BANK], F16, tag="r1b")
                nc.scalar.activation(r1a[:], pa[:], mybir.ActivationFunctionType.Relu,
                                     bias=b1t[:, :1], scale=1.0)
                nc.scalar.activation(r1b[:], pb[:], mybir.ActivationFunctionType.Relu,
                                     bias=b1t[:, :1], scale=1.0)
                for s in range(BANK // P):
                    pt = ps2.tile([128, 40], F32, tag="h2")
                    nc.tensor.matmul(out=pt[:], lhsT=r1a[:, s * P:(s + 1) * P],
                                     rhs=w2t[:, 0, :], start=True, stop=False)
                    nc.tensor.matmul(out=pt[:], lhsT=r1b[:, s * P:(s + 1) * P],
                                     rhs=w2t[:, 1, :], start=False, stop=True)
                    he = ev.tile([128, 128], F16, tag="he")
                    nc.vector.memset(he[:], 0.0)
                    nc.vector.tensor_copy(he[:, :40], pt[:])
                    r0 = b * BANK + s * P
                    nc.sync.dma_start(out=h2[r0:r0 + P, :], in_=he[:])
    nc.compile()
    return nc


def build_run3(plan_s, plan_k):
    """L2 aggregation (both graphs) + R2 + logits + log_softmax for one core."""
    nc = bacc.Bacc(None, target_bir_lowering=False)
    tb = nc.dram_tensor("tb", [NPAD, 128], F16, kind="ExternalInput")
    sa = nc.dram_tensor("sa", [128, plan_s["n_stiles"] * STILE], F16, kind="ExternalInput")
    ia = nc.dram_tensor("ia", [128, plan_s["n_itiles"] * ITILE], I16, kind="ExternalInput")
    sk = nc.dram_tensor("sk", [128, plan_k["n_stiles"] * STILE], F16, kind="ExternalInput")
    ik = nc.dram_tensor("ik", [128, plan_k["n_itiles"] * ITILE], I16, kind="ExternalInput")
    wlt = nc.dram_tensor("wlt", [104, 40], F16, kind="ExternalInput")
    b2v = nc.dram_tensor("b2v", [128, 1], F32, kind="ExternalInput")
    blr = nc.dram_tensor("blr", [128, 40], F32, kind="ExternalInput")
    out = nc.dram_tensor("out", [SH, 40], F32, kind="ExternalOutput")
    plan_s["dram_s"], plan_s["dram_i"] = sa, ia
    plan_k["dram_s"], plan_k["dram_i"] = sk, ik
    with tile.TileContext(nc) as tc:
        with (
            tc.tile_pool(name="const", bufs=1) as cp,
            tc.tile_pool(name="sb", bufs=3) as sb,
            tc.tile_pool(name="r2", bufs=2) as r2p,
            tc.tile_pool(name="ev", bufs=4) as ev,
            tc.tile_pool(name="ps", bufs=2, space="PSUM") as ps,
            tc.tile_pool(name="ps2", bufs=2, space="PSUM") as ps2,
        ):
            wltt = cp.tile([104, 40], F16)
            nc.sync.dma_start(out=wltt[:], in_=wlt[:])
            b2t = cp.tile([128, 1], F32)
            nc.sync.dma_start(out=b2t[:], in_=b2v[:])
            blt = cp.tile([128, 40], F32)
            nc.sync.dma_start(out=blt[:], in_=blr[:])

            es = AggEmitter(nc, sb, ps, plan_s, tb, 40, "s")
            ek = AggEmitter(nc, sb, ps, plan_k, tb, 40, "k")
            for b in range(NBANK):
                pa = es.emit_bank(b)
                pb = ek.emit_bank(b)
                r2t = r2p.tile([104, BANK], F16, tag="r2")
                nc.vector.tensor_scalar_add(r2t[0:40, :], pa[:40, :], b2t[:40, :1])
                nc.vector.tensor_scalar_add(r2t[64:104, :], pb[:40, :], b2t[:40, :1])
                for s in range(BANK // P):
                    pt = ps2.tile([128, 40], F32, tag="lg")
                    nc.tensor.matmul(out=pt[:], lhsT=r2t[:, s * P:(s + 1) * P],
                                     rhs=wltt[:], start=True, stop=True)
                    lg = ev.tile([128, 40], F32, tag="lg_sb")
                    nc.vector.tensor_add(lg[:], pt[:], blt[:])
                    mx = ev.tile([128, 1], F32, tag="mx")
                    nc.vector.tensor_reduce(mx[:], lg[:], mybir.AxisListType.X,
                                            mybir.AluOpType.max)
                    mxn = ev.tile([128, 1], F32, tag="mxn")
                    nc.vector.tensor_scalar_mul(mxn[:], mx[:], -1.0)
                    ex = ev.tile([128, 40], F32, tag="ex")
                    sm = ev.tile([128, 1], F32, tag="sm")
                    nc.scalar.activation(ex[:], lg[:], mybir.ActivationFunctionType.Exp,
                                         bias=mxn[:, :1], scale=1.0,
                                         accum_out=sm[:, :1])
                    ls = ev.tile([128, 1], F32, tag="ls")
                    nc.scalar.activation(ls[:], sm[:], mybir.ActivationFunctionType.Ln)
                    c = ev.tile([128, 1], F32, tag="c")
                    nc.vector.tensor_add(c[:], mx[:], ls[:])
                    fin = ev.tile([128, 40], F32, tag="fin")
                    nc.vector.tensor_scalar_sub(fin[:], lg[:], c[:, :1])
                    r0 = b * BANK + s * P
                    nc.sync.dma_start(out=out[r0:r0 + P, :], in_=fin[:])
    nc.compile()
    return nc




class DeviceProgram:
    def __init__(self, nc, device):
        install_neuronx_cc_hook()
        self.nc = nc
        self.device = device
        partition_name = nc.partition_id_tensor.name if nc.partition_id_tensor else None
        in_names, out_names, out_avals, zero_outs = [], [], [], []
        for alloc in nc.m.functions[0].allocations:
            if not isinstance(alloc, mybir.MemoryLocationSet):
                continue
            name = alloc.memorylocations[0].name
            if alloc.kind == "ExternalInput":
                if name != partition_name:
                    in_names.append(name)
            elif alloc.kind == "ExternalOutput":
                shape = tuple(alloc.tensor_shape)
                dtype = mybir.dt.np(alloc.dtype)
                out_names.append(name)
                out_avals.append(jax.core.ShapedArray(shape, dtype))
                zero_outs.append(np.zeros(shape, dtype))
        self.in_names = list(in_names)
        self.out_names = out_names
        self.out_avals = out_avals
        self.zero_outs = zero_outs
        n_params = len(in_names)
        all_names = in_names + out_names + ([partition_name] if partition_name else [])
        self.n_params = n_params
        donate = tuple(range(n_params, n_params + len(out_names)))

        def _body(*args):
            operands = list(args)
            if partition_name is not None:
                operands.append(partition_id_tensor())
            outs = _bass_exec_p.bind(
                *operands,
                out_avals=tuple(out_avals),
                in_names=tuple(all_names),
                out_names=tuple(out_names),
                lowering_input_output_aliases=(),
                sim_require_finite=True,
                sim_require_nnan=True,
                nc=nc,
            )
            return tuple(outs)

        self.fn = jax.jit(_body, donate_argnums=donate, keep_unused=True)
        self.dev_inputs = None

    def upload(self, in_map):
        arrs = [np.asarray(in_map[n]) for n in self.in_names]
        self.dev_inputs = [jax.device_put(a, self.device) for a in arrs]

    def call(self):
        """Run once; returns dict of np outputs. Re-creates donated zero outs."""
        zo = [jax.device_put(z, self.device) for z in self.zero_outs]
        outs = self.fn(*self.dev_inputs, *zo)
        return outs

    def results(self, outs):
        return {n: np.asarray(o) for n, o in zip(self.out_names, outs)}




N_CORES = 8
N_REAL = 100000


def host_prep(edge_index, edge_index_knn):
    dinv_s = degrees_dinv(edge_index)
    dinv_k = degrees_dinv(edge_index_knn)
    plans_s, plans_k = [], []
    for core in range(N_CORES):
        plans_s.append(pack_streams(build_shard_plan(edge_index, dinv_s, core)))
        plans_k.append(pack_streams(build_shard_plan(edge_index_knn, dinv_k, core)))
    return plans_s, plans_k


def build_programs(plans_s, plans_k, verbose=True):
    t0 = time.time()
    nc1 = build_run1()
    if verbose:
        print(f"[build] run1 {time.time()-t0:.1f}s", flush=True)
    nc2s, nc3s = [], []
    for core in range(N_CORES):
        t = time.time()
        nc2s.append(build_run2(plans_s[core], plans_k[core]))
        nc3s.append(build_run3(plans_s[core], plans_k[core]))
        if verbose:
            print(f"[build] core {core} run2+run3 {time.time()-t:.1f}s", flush=True)
    return nc1, nc2s, nc3s


def _parallel(fns):
    outs = [None] * len(fns)
    errs = []

    def wrap(i):
        try:
            outs[i] = fns[i]()
        except Exception as e:  # noqa: BLE001
            import traceback
            errs.append((i, e, traceback.format_exc()))

    ts = [threading.Thread(target=wrap, args=(i,)) for i in range(len(fns))]
    for t in ts:
        t.start()
    for t in ts:
        t.join()
    if errs:
        raise RuntimeError(f"thread errors: {[(i, tb) for i, _, tb in errs]}")
    return outs


class Pipeline:
    def __init__(self, inputs, verbose=True):
        self.v = verbose
        self.inputs = inputs
        self.devices = jax.devices()[:N_CORES]
        t0 = time.time()
        self.plans_s, self.plans_k = host_prep(
            inputs["edge_index"], inputs["edge_index_knn"])
        if verbose:
            print(f"[prep] plans {time.time()-t0:.1f}s", flush=True)
        nc1, nc2s, nc3s = build_programs(self.plans_s, self.plans_k, verbose)
        t0 = time.time()
        self.p1 = [DeviceProgram(nc1, self.devices[i]) for i in range(N_CORES)]
        self.p2 = [DeviceProgram(nc2s[i], self.devices[i]) for i in range(N_CORES)]
        self.p3 = [DeviceProgram(nc3s[i], self.devices[i]) for i in range(N_CORES)]
        if verbose:
            print(f"[build] DevicePrograms {time.time()-t0:.1f}s", flush=True)
        self._prepare_inputs()

    def _prepare_inputs(self):
        ins = self.inputs
        x = np.asarray(ins["x"])
        W1 = np.asarray(ins["W1"]).astype(np.float16)
        W2 = np.asarray(ins["W2"]).astype(np.float16)
        Wlin = np.asarray(ins["Wlin"]).astype(np.float16)
        b1 = np.asarray(ins["b1"]).astype(np.float32)
        b2 = np.asarray(ins["b2"]).astype(np.float32)
        blin = np.asarray(ins["blin"]).astype(np.float32)

        w1p = np.zeros((512, 128), np.float16)
        w1p[:500] = W1
        b1v = b1[:, None]
        b2v = np.zeros((128, 1), np.float32)
        b2v[:40, 0] = b2
        blr = np.tile(blin[None, :], (128, 1)).astype(np.float32)
        wlt = np.zeros((104, 40), np.float16)
        wlt[0:40] = Wlin.T[0:40]
        wlt[64:104] = Wlin.T[40:80]

        self.run1_maps = []
        for i in range(N_CORES):
            xs = np.zeros((SH, 512), np.float16)
            lo, hi = i * SH, min((i + 1) * SH, N_REAL)
            if hi > lo:
                xs[:hi - lo, :500] = x[lo:hi].astype(np.float16)
            self.run1_maps.append({"xT": np.ascontiguousarray(xs.T), "w1": w1p})
        self.consts2 = {"w2": W2, "b1v": b1v}
        self.consts3 = {"wlt": wlt, "b2v": b2v, "blr": blr}

    def run(self, time_it=False):
        v = self.v
        t0 = time.time()
        # ---- run 1
        for i in range(N_CORES):
            self.p1[i].upload(self.run1_maps[i])
        outs1 = _parallel([self.p1[i].call for i in range(N_CORES)])
        h_shards = [self.p1[i].results(outs1[i])["h"] for i in range(N_CORES)]
        table1 = np.concatenate(h_shards, axis=0)  # [NPAD, 128] f16
        if v:
            print(f"[run1] done {time.time()-t0:.1f}s", flush=True)

        # ---- run 2
        t0 = time.time()
        for i in range(N_CORES):
            m = {"tb": table1,
                 "sa": self.plans_s[i]["s_arr"], "ia": self.plans_s[i]["i_arr"],
                 "sk": self.plans_k[i]["s_arr"], "ik": self.plans_k[i]["i_arr"],
                 **self.consts2}
            self.p2[i].upload(m)
        outs2 = _parallel([self.p2[i].call for i in range(N_CORES)])
        h2_shards = [self.p2[i].results(outs2[i])["h2"] for i in range(N_CORES)]
        table2 = np.concatenate(h2_shards, axis=0)  # [NPAD, 128] f16
        if v:
            print(f"[run2] done {time.time()-t0:.1f}s", flush=True)

        # ---- run 3
        t0 = time.time()
        for i in range(N_CORES):
            m = {"tb": table2,
                 "sa": self.plans_s[i]["s_arr"], "ia": self.plans_s[i]["i_arr"],
                 "sk": self.plans_k[i]["s_arr"], "ik": self.plans_k[i]["i_arr"],
                 **self.consts3}
            self.p3[i].upload(m)
        outs3 = _parallel([self.p3[i].call for i in range(N_CORES)])
        out_shards = [self.p3[i].results(outs3[i])["out"] for i in range(N_CORES)]
        result = np.concatenate(out_shards, axis=0)[:N_REAL]
        if v:
            print(f"[run3] done {time.time()-t0:.1f}s", flush=True)

        times = None
        if time_it:
            times = self.time_runs()
        return result, times

    def time_runs(self, reps=5):
        """Concurrent repeat timing per run; returns dict of per-run best wall
        seconds (all 8 devices running concurrently)."""
        times = {}
        for name, progs in (("run1", self.p1), ("run2", self.p2), ("run3", self.p3)):
            best = float("inf")
            for _ in range(reps):
                barrier = threading.Barrier(N_CORES + 1)
                done = []

                def worker(p):
                    barrier.wait()
                    o = p.call()
                    jax.block_until_ready(o)
                    done.append(o)

                ts = [threading.Thread(target=worker, args=(p,)) for p in progs]
                for t in ts:
                    t.start()
                barrier.wait()
                t0 = time.time()
                for t in ts:
                    t.join()
                best = min(best, time.time() - t0)
            times[name] = best
        return times

_PIPELINE_CACHE = {}


def kernel(**inputs):
    key = "singleton"
    pl = _PIPELINE_CACHE.get(key)
    if pl is None or pl.graph_key != _graph_key(inputs):
        pl = Pipeline(inputs, verbose=False)
        pl.graph_key = _graph_key(inputs)
        _PIPELINE_CACHE[key] = pl
    else:
        pl.inputs = inputs
        pl._prepare_inputs()
    out, _ = pl.run(time_it=False)
    return out.astype(np.float32)


def _graph_key(inputs):
    ei = np.asarray(inputs["edge_index"])
    ek = np.asarray(inputs["edge_index_knn"])
    return (ei.shape, ek.shape, int(ei[:, 0].sum()), int(ei[:, -1].sum()),
            int(ek[:, 0].sum()), int(ek[:, -1].sum()))



# revision 2
# speedup vs baseline: 1.1888x; 1.1888x over previous
"""Self-contained Trainium2 Bass kernel for the 2-layer dual-graph GCN
(nn_GCN0100). Accepts FULL inputs, returns FULL output.

Strategy: node-sharded across 8 NeuronCores, 3 SPMD-style launches:
  run1: h1 = x @ W1 per shard, emitted fp8 (table1)
  run2: layer-1 gather/segment-sum over both graphs (dma_gather of fp8
        128-B rows + on-device one-hot S build + matmul into PSUM),
        ReLU+bias, h2 = R1 @ W2 -> fp16 table2 shard
  run3: layer-2 gather/segment-sum (80-B partial-row fp16 gathers),
        logits, log_softmax
Host assembles the full tables between launches (free halo exchange) and
does index-only graph preprocessing; all FLOPs / per-edge data movement
run on device.
"""
import threading
import time
import numpy as np
import jax
import concourse.bass as bass
import concourse.mybir as mybir
import concourse.tile as tile
import concourse.ap_utils as ap_utils
from concourse import bacc
from concourse.bass2jax import _bass_exec_p, partition_id_tensor, install_neuronx_cc_hook


P = 128
SH = 12800          # shard size (102400 / 8)
NPAD = 102400       # padded node count
CH = 25600          # gather chunk rows (int16 idx limit)
NCHUNK = NPAD // CH  # 4
BANK = 512          # PSUM bank slots
NBANK = SH // BANK   # 25
NGRP = (NBANK + 1) // 2  # 13 bank groups of 2 (last has 1)
N_CORES = 8
N_REAL = 100000

F8 = mybir.dt.float8e4
F16 = mybir.dt.float16
F32 = mybir.dt.float32
I16 = mybir.dt.int16


def raw_dma_gather(g, out_ap, in_ap, idxs_ap, num_idxs, elem_size, elem_step):
    """dma_gather with elem_size < row stride (partial-row reads).

    Same encoding as BassGpSimd.dma_gather (non-transpose, DRAM source)
    minus the elem_size%256 assert: the real constraint is that the row
    STRIDE is a multiple of 256B; the read size per descriptor may be
    smaller (verified on hardware)."""
    assert idxs_ap.dtype == mybir.dt.int16
    assert in_ap.dtype == out_ap.dtype
    assert ap_utils.ap_is_contiguous(in_ap.ap[1:])
    assert ap_utils.ap_is_contiguous(out_ap.ap[1:])
    assert ap_utils.ap_is_contiguous(idxs_ap.ap[1:])
    assert in_ap.ap[-1][1] == out_ap.ap[-1][1] == elem_size
    assert in_ap.ap[0][0] == elem_step
    stride_bytes = elem_step * mybir.dt.size(in_ap.dtype)
    assert stride_bytes % 256 == 0 and stride_bytes // 256 < 256
    _in_ap = g.lower_ap_dma(in_ap, for_custom_bir_dma=True)
    _idxs_ap = g.lower_ap(idxs_ap)
    _out_ap = g.lower_ap(out_ap)
    return g.add_instruction(mybir.InstDMAGatherAnt(
        name=g.bass.get_next_instruction_name(),
        ins=[*_in_ap, _idxs_ap, g.lower_val_access(g.to_reg(num_idxs))],
        outs=[_out_ap], transpose=False, num_idxs=num_idxs,
        elem_size=elem_size, stride_bytes_256=stride_bytes // 256,
        gen_mode=0, single_packet=False, queue_num=0,
        sbuf_tokens_per_rank=0, sbuf_free_dim_per_rank=0,
        sbuf_free_dim_pad_per_rank=0, sbuf_byte_offset=0))


def degrees_dinv(edge_index, n=N_REAL):
    deg = np.bincount(np.asarray(edge_index[1]), minlength=n).astype(np.float64) + 1.0
    return (1.0 / np.sqrt(deg)).astype(np.float32)


def build_plan(edge_index, edge_index_knn, core):
    """One merged plan per core, shared by run2 and run3 (same edges).

    Returns dict with device streams (ia / ws / wn) and call/window
    metadata for the emit loop."""
    dinv_s = degrees_dinv(edge_index)
    dinv_k = degrees_dinv(edge_index_knn)
    n0, n1 = core * SH, core * SH + SH
    rows, slots, norms, graphs = [], [], [], []
    for gi, (ei, dinv) in enumerate(((edge_index, dinv_s), (edge_index_knn, dinv_k))):
        row = np.asarray(ei[0]).astype(np.int64)
        col = np.asarray(ei[1]).astype(np.int64)
        m = (col >= n0) & (col < n1)
        row, col = row[m], col[m]
        selfn = np.arange(n0, min(n1, N_REAL), dtype=np.int64)
        row = np.concatenate([row, selfn])
        col = np.concatenate([col, selfn])
        rows.append(row)
        slots.append((col - n0).astype(np.int32))
        norms.append((dinv[row] * dinv[col]).astype(np.float32))
        graphs.append(np.full(len(row), gi, np.int8))
    row = np.concatenate(rows)
    slot = np.concatenate(slots)
    norm = np.concatenate(norms)
    graph = np.concatenate(graphs)
    bank = slot >> 9
    grp = bank >> 1
    chunk = (row // CH).astype(np.int32)

    order = np.lexsort((slot, bank, graph, chunk, grp))
    row, slot, norm, graph, bank, grp, chunk = (
        a[order] for a in (row, slot, norm, graph, bank, grp, chunk))

    # cell = (grp, chunk, graph, bank); pad each to x128
    key = ((grp.astype(np.int64) * NCHUNK + chunk) * 2 + graph) * NBANK + bank
    uniq, starts = np.unique(key, return_index=True)
    starts = np.sort(starts)
    bounds = list(starts) + [len(key)]

    calls = []       # dicts: grp, chunk, idx (int16 local), windows list
    cur_call = None
    s_blocks = []    # per-window [128, B] fp32 one-hot*norm (quantized later)
    nwin = 0
    for s, e in zip(bounds[:-1], bounds[1:]):
        g_, b_, c_, gr_ = int(graph[s]), int(bank[s]), int(chunk[s]), int(grp[s])
        r_, sl_, nm_ = row[s:e], slot[s:e], norm[s:e]
        pad = (-len(r_)) % P
        if pad:
            r_ = np.concatenate([r_, np.full(pad, r_[-1], np.int64)])
            sl_ = np.concatenate([sl_, np.full(pad, sl_[-1], np.int32)])
            nm_ = np.concatenate([nm_, np.zeros(pad, np.float32)])
        if cur_call is None or cur_call["grp"] != gr_ or cur_call["chunk"] != c_:
            cur_call = {"grp": gr_, "chunk": c_, "idx": [], "windows": []}
            calls.append(cur_call)
        base = len(cur_call["idx"])
        cur_call["idx"].extend((r_ - c_ * CH).astype(np.int16))
        nw = len(r_) // P
        for w in range(nw):
            ssl = sl_[w * P:(w + 1) * P]
            snm = nm_[w * P:(w + 1) * P]
            smin = int(ssl.min())
            B = int(ssl.max()) - smin + 1
            S = np.zeros((P, B), np.float32)
            S[np.arange(P), ssl - smin] = snm
            cur_call["windows"].append({
                "wslot": base // P + w, "graph": g_, "bank": b_,
                "smin": smin - b_ * BANK, "B": B, "wid": nwin})
            s_blocks.append(S)
            nwin += 1
    # idx stream: per call, wrapped [16, n/16] replicated to 128 partitions
    itot = sum(len(c["idx"]) // 16 for c in calls)
    ia = np.zeros((P, itot), np.int16)
    off = 0
    for c in calls:
        idx = np.asarray(c["idx"], np.int16)
        wrapped = idx.reshape(-1, 16).T
        ia[:, off:off + wrapped.shape[1]] = np.tile(wrapped, (8, 1))
        c["i_col"] = off
        c["n"] = len(idx)
        off += wrapped.shape[1]
    return {"calls": calls, "ia": ia, "s_blocks": s_blocks, "NW": nwin}


# --------------------------- device programs ---------------------------

def build_run1():
    """h1 = x @ W1 for one shard; output fp8 (identical for all cores).
    Inputs: xT [512, SH] f16, w1 [512, 128] f16. Output: h [SH, 128] f8."""
    nc = bacc.Bacc(None, target_bir_lowering=False)
    xT = nc.dram_tensor("xT", [512, SH], F16, kind="ExternalInput")
    w1 = nc.dram_tensor("w1", [512, 128], F16, kind="ExternalInput")
    h = nc.dram_tensor("h", [SH, 128], F8, kind="ExternalOutput")
    KX = 4
    with tile.TileContext(nc) as tc:
        with (
            tc.tile_pool(name="const", bufs=1) as cp,
            tc.tile_pool(name="sb", bufs=3) as sb,
            tc.tile_pool(name="ev", bufs=4) as ev,
            tc.tile_pool(name="ps", bufs=2, space="PSUM") as ps,
        ):
            w1t = cp.tile([128, KX, 128], F16)
            for kc in range(KX):
                nc.sync.dma_start(out=w1t[:, kc, :], in_=w1[kc * 128:(kc + 1) * 128, :])
            for t in range(SH // 512):
                xt = sb.tile([128, KX, 512], F16, tag="xt")
                for kc in range(KX):
                    nc.sync.dma_start(
                        out=xt[:, kc, :],
                        in_=xT[kc * 128:(kc + 1) * 128, t * 512:(t + 1) * 512])
                for s in range(4):
                    pt = ps.tile([128, 128], F32, tag="h")
                    for kc in range(KX):
                        nc.tensor.matmul(
                            out=pt[:], lhsT=xt[:, kc, s * 128:(s + 1) * 128],
                            rhs=w1t[:, kc, :], start=(kc == 0), stop=(kc == KX - 1))
                    he = ev.tile([128, 128], F8, tag="he")
                    if s % 2 == 0:
                        nc.vector.tensor_copy(he[:], pt[:])
                    else:
                        nc.scalar.activation(he[:], pt[:],
                                             mybir.ActivationFunctionType.Copy)
                    nc.sync.dma_start(
                        out=h[(t * 4 + s) * 128:(t * 4 + s + 1) * 128, :], in_=he[:])
    nc.compile()
    return nc


def emit_agg_groups(nc, tc, pools, plan, table, elem, elem_step, tdtype,
                    ia, sa, consume_group):
    """Shared run2/run3 emit loop: per bank-group gather + aggregate, then
    call consume_group(grp, banks, psum_tiles) to drain."""
    cp, sb, sp, ps = pools
    calls_by_grp = {}
    for c in plan["calls"]:
        calls_by_grp.setdefault(c["grp"], []).append(c)
    for grp in range(NGRP):
        banks = [2 * grp] if 2 * grp == NBANK - 1 else [2 * grp, 2 * grp + 1]
        pt = {}
        for b in banks:
            for g in range(2):
                t_ = ps.tile([128, BANK], F32, tag=f"acc{g}{b & 1}")
                nc.vector.memset(t_[:], 0.0)
                pt[(g, b)] = t_
        slo, shi = plan["grp_s_range"][grp]
        st = None
        if shi > slo:
            st = sp.tile([128, shi - slo], F8, tag="st")
            nc.sync.dma_start(out=st[:], in_=sa[:, slo:shi])
        for call in calls_by_grp.get(grp, []):
            n = call["n"]
            it = sb.tile([128, n // 16], I16, tag="it")
            nc.sync.dma_start(out=it[:], in_=ia[:, call["i_col"]:call["i_col"] + n // 16])
            gt = sb.tile([128, n // P, elem], tdtype, tag="gt")
            c0 = call["chunk"] * CH
            raw_dma_gather(nc.gpsimd, gt[:],
                           table[c0:c0 + CH, 0:elem], it[:], n, elem, elem_step)
            for w in call["windows"]:
                B = w["B"]
                p_ = pt[(w["graph"], w["bank"])]
                nc.tensor.matmul(
                    out=p_[:elem, w["smin"]:w["smin"] + B],
                    lhsT=gt[:, w["wslot"], :],
                    rhs=st[:, w["s_col"]:w["s_col"] + B],
                    start=False, stop=True, skip_group_check=True)
        consume_group(grp, banks, pt)


def annotate_plan(plan):
    """Pack per-window S blocks into a per-group fp8 stream (4-col aligned
    slices) and record group S-column ranges."""
    import ml_dtypes
    grp_cols = {g: 0 for g in range(NGRP)}
    for c in plan["calls"]:
        for w in c["windows"]:
            w["s_col"] = grp_cols[c["grp"]]
            grp_cols[c["grp"]] += (w["B"] + 3) & ~3
    starts = {}
    off = 0
    for g in range(NGRP):
        starts[g] = off
        off += grp_cols[g]
    SBTOT = max(off, 4)
    s_arr = np.zeros((P, SBTOT), ml_dtypes.float8_e4m3)
    for c in plan["calls"]:
        for w in c["windows"]:
            S = plan["s_blocks"][w["wid"]]
            c0 = starts[c["grp"]] + w["s_col"]
            s_arr[:, c0:c0 + w["B"]] = S.astype(ml_dtypes.float8_e4m3)
    plan["s_arr"] = s_arr
    plan["grp_s_range"] = {g: (starts[g], starts[g] + grp_cols[g])
                           for g in range(NGRP)}
    del plan["s_blocks"]
    return plan


def build_run2(plan):
    """L1 aggregation (both graphs) + R1 + h2 = R1 @ W2 for one core."""
    nc = bacc.Bacc(None, target_bir_lowering=False)
    t1 = nc.dram_tensor("t1", [NPAD, 256], F8, kind="ExternalInput")
    ia = nc.dram_tensor("ia", [P, plan["ia"].shape[1]], I16, kind="ExternalInput")
    sa = nc.dram_tensor("sa", [P, plan["s_arr"].shape[1]], F8, kind="ExternalInput")
    w2 = nc.dram_tensor("w2", [256, 40], F16, kind="ExternalInput")
    b1v = nc.dram_tensor("b1v", [128, 1], F32, kind="ExternalInput")
    h2 = nc.dram_tensor("h2", [SH, 40], F8, kind="ExternalOutput")
    with tile.TileContext(nc) as tc:
        with (
            tc.tile_pool(name="const", bufs=1) as cp,
            tc.tile_pool(name="sb", bufs=3) as sb,
            tc.tile_pool(name="sp", bufs=4) as sp,
            tc.tile_pool(name="r1", bufs=2) as r1p,
            tc.tile_pool(name="ev", bufs=4) as ev,
            tc.tile_pool(name="ps", bufs=1, space="PSUM") as ps,
            tc.tile_pool(name="ps2", bufs=2, space="PSUM") as ps2,
        ):
            w2t = cp.tile([128, 2, 40], F16)
            for kc in range(2):
                nc.sync.dma_start(out=w2t[:, kc, :], in_=w2[kc * 128:(kc + 1) * 128, :])
            b1t = cp.tile([128, 1], F32)
            nc.sync.dma_start(out=b1t[:], in_=b1v[:])

            def consume(grp, banks, pt):
                for b in banks:
                    r1a = r1p.tile([128, BANK], F16, tag="r1a")
                    r1b = r1p.tile([128, BANK], F16, tag="r1b")
                    nc.scalar.activation(r1a[:], pt[(0, b)][:],
                                         mybir.ActivationFunctionType.Relu,
                                         bias=b1t[:, :1], scale=1.0)
                    nc.scalar.activation(r1b[:], pt[(1, b)][:],
                                         mybir.ActivationFunctionType.Relu,
                                         bias=b1t[:, :1], scale=1.0)
                    for s in range(BANK // P):
                        p2 = ps2.tile([128, 40], F32, tag="h2")
                        nc.tensor.matmul(out=p2[:], lhsT=r1a[:, s * P:(s + 1) * P],
                                         rhs=w2t[:, 0, :], start=True, stop=False)
                        nc.tensor.matmul(out=p2[:], lhsT=r1b[:, s * P:(s + 1) * P],
                                         rhs=w2t[:, 1, :], start=False, stop=True)
                        he = ev.tile([128, 40], F8, tag="he")
                        if s % 2 == 0:
                            nc.vector.tensor_copy(he[:], p2[:])
                        else:
                            nc.scalar.activation(he[:], p2[:],
                                                 mybir.ActivationFunctionType.Copy)
                        r0 = b * BANK + s * P
                        nc.sync.dma_start(out=h2[r0:r0 + P, :], in_=he[:])

            emit_agg_groups(nc, tc, (cp, sb, sp, ps), plan, t1, 128, 256, F8,
                            ia, sa, consume)
    nc.compile()
    return nc


def build_run3(plan):
    """L2 aggregation (both graphs) + logits + log_softmax for one core."""
    nc = bacc.Bacc(None, target_bir_lowering=False)
    t2 = nc.dram_tensor("t2", [NPAD, 256], F8, kind="ExternalInput")
    ia = nc.dram_tensor("ia", [P, plan["ia"].shape[1]], I16, kind="ExternalInput")
    sa = nc.dram_tensor("sa", [P, plan["s_arr"].shape[1]], F8, kind="ExternalInput")
    wlt = nc.dram_tensor("wlt", [104, 40], F16, kind="ExternalInput")
    b2v = nc.dram_tensor("b2v", [128, 1], F32, kind="ExternalInput")
    blr = nc.dram_tensor("blr", [128, 40], F32, kind="ExternalInput")
    out = nc.dram_tensor("out", [SH, 40], F32, kind="ExternalOutput")
    with tile.TileContext(nc) as tc:
        with (
            tc.tile_pool(name="const", bufs=1) as cp,
            tc.tile_pool(name="sb", bufs=3) as sb,
            tc.tile_pool(name="sp", bufs=4) as sp,
            tc.tile_pool(name="r2", bufs=2) as r2p,
            tc.tile_pool(name="ev", bufs=6) as ev,
            tc.tile_pool(name="ps", bufs=1, space="PSUM") as ps,
            tc.tile_pool(name="ps2", bufs=2, space="PSUM") as ps2,
        ):
            wltt = cp.tile([104, 40], F16)
            nc.sync.dma_start(out=wltt[:], in_=wlt[:])
            b2t = cp.tile([128, 1], F32)
            nc.sync.dma_start(out=b2t[:], in_=b2v[:])
            blt = cp.tile([128, 40], F32)
            nc.sync.dma_start(out=blt[:], in_=blr[:])

            def consume(grp, banks, pt):
                for b in banks:
                    r2t = r2p.tile([104, BANK], F16, tag="r2")
                    nc.vector.tensor_scalar_add(r2t[0:40, :], pt[(0, b)][:40, :],
                                                b2t[:40, :1])
                    nc.vector.tensor_scalar_add(r2t[64:104, :], pt[(1, b)][:40, :],
                                                b2t[:40, :1])
                    for s in range(BANK // P):
                        p2 = ps2.tile([128, 40], F32, tag="lg")
                        nc.tensor.matmul(out=p2[:], lhsT=r2t[:, s * P:(s + 1) * P],
                                         rhs=wltt[:], start=True, stop=True)
                        lg = ev.tile([128, 40], F32, tag="lg_sb")
                        nc.vector.tensor_add(lg[:], p2[:], blt[:])
                        mx = ev.tile([128, 1], F32, tag="mx")
                        nc.vector.tensor_reduce(mx[:], lg[:], mybir.AxisListType.X,
                                                mybir.AluOpType.max)
                        mxn = ev.tile([128, 1], F32, tag="mxn")
                        nc.vector.tensor_scalar_mul(mxn[:], mx[:], -1.0)
                        ex = ev.tile([128, 40], F32, tag="ex")
                        sm = ev.tile([128, 1], F32, tag="sm")
                        nc.scalar.activation(ex[:], lg[:],
                                             mybir.ActivationFunctionType.Exp,
                                             bias=mxn[:, :1], scale=1.0,
                                             accum_out=sm[:, :1])
                        ls = ev.tile([128, 1], F32, tag="ls")
                        nc.scalar.activation(ls[:], sm[:],
                                             mybir.ActivationFunctionType.Ln)
                        c_ = ev.tile([128, 1], F32, tag="c")
                        nc.vector.tensor_add(c_[:], mx[:], ls[:])
                        fin = ev.tile([128, 40], F32, tag="fin")
                        nc.vector.tensor_scalar_sub(fin[:], lg[:], c_[:, :1])
                        r0 = b * BANK + s * P
                        nc.sync.dma_start(out=out[r0:r0 + P, :], in_=fin[:])

            emit_agg_groups(nc, tc, (cp, sb, sp, ps), plan, t2, 40, 256, F8,
                            ia, sa, consume)
    nc.compile()
    return nc


# --------------------------- execution harness ---------------------------

class DeviceProgram:
    def __init__(self, nc, device):
        install_neuronx_cc_hook()
        self.nc = nc
        self.device = device
        partition_name = nc.partition_id_tensor.name if nc.partition_id_tensor else None
        in_names, out_names, out_avals, zero_outs = [], [], [], []
        for alloc in nc.m.functions[0].allocations:
            if not isinstance(alloc, mybir.MemoryLocationSet):
                continue
            name = alloc.memorylocations[0].name
            if alloc.kind == "ExternalInput":
                if name != partition_name:
                    in_names.append(name)
            elif alloc.kind == "ExternalOutput":
                shape = tuple(alloc.tensor_shape)
                dtype = mybir.dt.np(alloc.dtype)
                out_names.append(name)
                out_avals.append(jax.core.ShapedArray(shape, dtype))
                zero_outs.append(np.zeros(shape, dtype))
        self.in_names = list(in_names)
        self.out_names = out_names
        self.out_avals = out_avals
        self.zero_outs = zero_outs
        n_params = len(in_names)
        all_names = in_names + out_names + ([partition_name] if partition_name else [])
        self.n_params = n_params
        donate = tuple(range(n_params, n_params + len(out_names)))

        def _body(*args):
            operands = list(args)
            if partition_name is not None:
                operands.append(partition_id_tensor())
            outs = _bass_exec_p.bind(
                *operands,
                out_avals=tuple(out_avals),
                in_names=tuple(all_names),
                out_names=tuple(out_names),
                lowering_input_output_aliases=(),
                sim_require_finite=True,
                sim_require_nnan=True,
                nc=nc,
            )
            return tuple(outs)

        self.fn = jax.jit(_body, donate_argnums=donate, keep_unused=True)
        self.dev_inputs = None

    def upload(self, in_map):
        arrs = [np.asarray(in_map[n]) for n in self.in_names]
        self.dev_inputs = [jax.device_put(a, self.device) for a in arrs]

    def call(self):
        zo = [jax.device_put(z, self.device) for z in self.zero_outs]
        outs = self.fn(*self.dev_inputs, *zo)
        return outs

    def results(self, outs):
        return {n: np.asarray(o) for n, o in zip(self.out_names, outs)}


def _parallel(fns):
    outs = [None] * len(fns)
    errs = []

    def wrap(i):
        try:
            outs[i] = fns[i]()
        except Exception as e:  # noqa: BLE001
            import traceback
            errs.append((i, e, traceback.format_exc()))

    ts = [threading.Thread(target=wrap, args=(i,)) for i in range(len(fns))]
    for t in ts:
        t.start()
    for t in ts:
        t.join()
    if errs:
        raise RuntimeError(f"thread errors: {[(i, tb) for i, _, tb in errs]}")
    return outs


class Pipeline:
    def __init__(self, inputs, verbose=True):
        self.v = verbose
        self.inputs = inputs
        self.devices = jax.devices()[:N_CORES]
        t0 = time.time()
        self.plans = [annotate_plan(build_plan(
            inputs["edge_index"], inputs["edge_index_knn"], c))
            for c in range(N_CORES)]
        if verbose:
            print(f"[prep] plans {time.time()-t0:.1f}s", flush=True)
        t0 = time.time()
        nc1 = build_run1()
        if verbose:
            print(f"[build] run1 {time.time()-t0:.1f}s", flush=True)
        nc2s, nc3s = [], []
        for core in range(N_CORES):
            t = time.time()
            nc2s.append(build_run2(self.plans[core]))
            nc3s.append(build_run3(self.plans[core]))
            if verbose:
                print(f"[build] core {core} run2+run3 {time.time()-t:.1f}s", flush=True)
        t0 = time.time()
        self.p1 = [DeviceProgram(nc1, self.devices[i]) for i in range(N_CORES)]
        self.p2 = [DeviceProgram(nc2s[i], self.devices[i]) for i in range(N_CORES)]
        self.p3 = [DeviceProgram(nc3s[i], self.devices[i]) for i in range(N_CORES)]
        if verbose:
            print(f"[build] DevicePrograms {time.time()-t0:.1f}s", flush=True)
        self._prepare_inputs()

    def _prepare_inputs(self):
        ins = self.inputs
        x = np.asarray(ins["x"])
        W1 = np.asarray(ins["W1"]).astype(np.float16)
        W2 = np.asarray(ins["W2"]).astype(np.float16)
        Wlin = np.asarray(ins["Wlin"]).astype(np.float16)
        b1 = np.asarray(ins["b1"]).astype(np.float32)
        b2 = np.asarray(ins["b2"]).astype(np.float32)
        blin = np.asarray(ins["blin"]).astype(np.float32)

        w1p = np.zeros((512, 128), np.float16)
        w1p[:500] = W1
        b1v = b1[:, None]
        b2v = np.zeros((128, 1), np.float32)
        b2v[:40, 0] = b2
        blr = np.tile(blin[None, :], (128, 1)).astype(np.float32)
        wlt = np.zeros((104, 40), np.float16)
        wlt[0:40] = Wlin.T[0:40]
        wlt[64:104] = Wlin.T[40:80]

        self.run1_maps = []
        for i in range(N_CORES):
            xs = np.zeros((SH, 512), np.float16)
            lo, hi = i * SH, min((i + 1) * SH, N_REAL)
            if hi > lo:
                xs[:hi - lo, :500] = x[lo:hi].astype(np.float16)
            self.run1_maps.append({"xT": np.ascontiguousarray(xs.T), "w1": w1p})
        self.consts2 = {"w2": W2, "b1v": b1v}
        self.consts3 = {"wlt": wlt, "b2v": b2v, "blr": blr}

    def run(self, time_it=False):
        v = self.v
        t0 = time.time()
        for i in range(N_CORES):
            self.p1[i].upload(self.run1_maps[i])
        outs1 = _parallel([self.p1[i].call for i in range(N_CORES)])
        h_shards = [self.p1[i].results(outs1[i])["h"] for i in range(N_CORES)]
        table1 = np.zeros((NPAD, 256), h_shards[0].dtype)
        table1[:, :128] = np.concatenate(h_shards, axis=0)
        if v:
            print(f"[run1] done {time.time()-t0:.1f}s", flush=True)

        t0 = time.time()
        for i in range(N_CORES):
            pl = self.plans[i]
            self.p2[i].upload({"t1": table1, "ia": pl["ia"], "sa": pl["s_arr"],
                               **self.consts2})
        outs2 = _parallel([self.p2[i].call for i in range(N_CORES)])
        h2_shards = [self.p2[i].results(outs2[i])["h2"] for i in range(N_CORES)]
        table2 = np.zeros((NPAD, 256), h2_shards[0].dtype)
        table2[:, :40] = np.concatenate(h2_shards, axis=0)
        if v:
            print(f"[run2] done {time.time()-t0:.1f}s", flush=True)

        t0 = time.time()
        for i in range(N_CORES):
            pl = self.plans[i]
            self.p3[i].upload({"t2": table2, "ia": pl["ia"], "sa": pl["s_arr"],
                               **self.consts3})
        outs3 = _parallel([self.p3[i].call for i in range(N_CORES)])
        out_shards = [self.p3[i].results(outs3[i])["out"] for i in range(N_CORES)]
        result = np.concatenate(out_shards, axis=0)[:N_REAL]
        if v:
            print(f"[run3] done {time.time()-t0:.1f}s", flush=True)

        times = None
        if time_it:
            times = self.time_runs()
        return result, times

    def time_runs(self, reps=5):
        times = {}
        for name, progs in (("run1", self.p1), ("run2", self.p2), ("run3", self.p3)):
            best = float("inf")
            for _ in range(reps):
                barrier = threading.Barrier(N_CORES + 1)
                done = []

                def worker(p):
                    barrier.wait()
                    o = p.call()
                    jax.block_until_ready(o)
                    done.append(o)

                ts = [threading.Thread(target=worker, args=(p,)) for p in progs]
                for t in ts:
                    t.start()
                barrier.wait()
                t0 = time.time()
                for t in ts:
                    t.join()
                best = min(best, time.time() - t0)
            times[name] = best
        return times


_PIPELINE_CACHE = {}


def kernel(**inputs):
    key = "singleton"
    pl = _PIPELINE_CACHE.get(key)
    if pl is None or pl.graph_key != _graph_key(inputs):
        pl = Pipeline(inputs, verbose=False)
        pl.graph_key = _graph_key(inputs)
        _PIPELINE_CACHE[key] = pl
    else:
        pl.inputs = inputs
        pl._prepare_inputs()
    out, _ = pl.run(time_it=False)
    return out.astype(np.float32)


def _graph_key(inputs):
    ei = np.asarray(inputs["edge_index"])
    ek = np.asarray(inputs["edge_index_knn"])
    return (ei.shape, ek.shape, int(ei[:, 0].sum()), int(ei[:, -1].sum()),
            int(ek[:, 0].sum()), int(ek[:, -1].sum()))


# revision 3
# speedup vs baseline: 1.2704x; 1.0686x over previous
"""Self-contained Trainium2 Bass kernel for the 2-layer dual-graph GCN
(nn_GCN0100). Accepts FULL inputs, returns FULL output.

Strategy: node-sharded across 8 NeuronCores, 3 SPMD-style launches:
  run1: h1 = x @ W1 per shard, emitted fp8 (table1)
  run2: layer-1 gather/segment-sum over both graphs (dma_gather of fp8
        128-B rows + on-device one-hot S build + matmul into PSUM),
        ReLU+bias, h2 = R1 @ W2 -> fp16 table2 shard
  run3: layer-2 gather/segment-sum (80-B partial-row fp16 gathers),
        logits, log_softmax
Host assembles the full tables between launches (free halo exchange) and
does index-only graph preprocessing; all FLOPs / per-edge data movement
run on device.
"""
import threading
import time
import numpy as np
import jax
import concourse.bass as bass
import concourse.mybir as mybir
import concourse.tile as tile
import concourse.ap_utils as ap_utils
from concourse import bacc
from concourse.bass2jax import _bass_exec_p, partition_id_tensor, install_neuronx_cc_hook


P = 128
SH = 12800          # shard size (102400 / 8)
NPAD = 102400       # padded node count
CH = 25600          # gather chunk rows (int16 idx limit)
NCHUNK = NPAD // CH  # 4
BANK = 512          # PSUM bank slots
NBANK = SH // BANK   # 25
NGRP = (NBANK + 1) // 2  # 13 bank groups of 2 (last has 1)
N_CORES = 8
N_REAL = 100000

F8 = mybir.dt.float8e4
F16 = mybir.dt.float16
F32 = mybir.dt.float32
I16 = mybir.dt.int16


def raw_dma_gather(g, out_ap, in_ap, idxs_ap, num_idxs, elem_size, elem_step):
    """dma_gather with elem_size < row stride (partial-row reads).

    Same encoding as BassGpSimd.dma_gather (non-transpose, DRAM source)
    minus the elem_size%256 assert: the real constraint is that the row
    STRIDE is a multiple of 256B; the read size per descriptor may be
    smaller (verified on hardware)."""
    assert idxs_ap.dtype == mybir.dt.int16
    assert in_ap.dtype == out_ap.dtype
    assert ap_utils.ap_is_contiguous(in_ap.ap[1:])
    assert ap_utils.ap_is_contiguous(out_ap.ap[1:])
    assert ap_utils.ap_is_contiguous(idxs_ap.ap[1:])
    assert in_ap.ap[-1][1] == out_ap.ap[-1][1] == elem_size
    assert in_ap.ap[0][0] == elem_step
    stride_bytes = elem_step * mybir.dt.size(in_ap.dtype)
    assert stride_bytes % 256 == 0 and stride_bytes // 256 < 256
    _in_ap = g.lower_ap_dma(in_ap, for_custom_bir_dma=True)
    _idxs_ap = g.lower_ap(idxs_ap)
    _out_ap = g.lower_ap(out_ap)
    return g.add_instruction(mybir.InstDMAGatherAnt(
        name=g.bass.get_next_instruction_name(),
        ins=[*_in_ap, _idxs_ap, g.lower_val_access(g.to_reg(num_idxs))],
        outs=[_out_ap], transpose=False, num_idxs=num_idxs,
        elem_size=elem_size, stride_bytes_256=stride_bytes // 256,
        gen_mode=0, single_packet=False, queue_num=0,
        sbuf_tokens_per_rank=0, sbuf_free_dim_per_rank=0,
        sbuf_free_dim_pad_per_rank=0, sbuf_byte_offset=0))


def degrees_dinv(edge_index, n=N_REAL):
    deg = np.bincount(np.asarray(edge_index[1]), minlength=n).astype(np.float64) + 1.0
    return (1.0 / np.sqrt(deg)).astype(np.float32)


def build_plan(edge_index, edge_index_knn, core):
    """One merged plan per core, shared by run2 and run3 (same edges).

    Returns dict with device streams (ia / ws / wn) and call/window
    metadata for the emit loop."""
    dinv_s = degrees_dinv(edge_index)
    dinv_k = degrees_dinv(edge_index_knn)
    n0, n1 = core * SH, core * SH + SH
    rows, slots, norms, graphs = [], [], [], []
    for gi, (ei, dinv) in enumerate(((edge_index, dinv_s), (edge_index_knn, dinv_k))):
        row = np.asarray(ei[0]).astype(np.int64)
        col = np.asarray(ei[1]).astype(np.int64)
        m = (col >= n0) & (col < n1)
        row, col = row[m], col[m]
        selfn = np.arange(n0, min(n1, N_REAL), dtype=np.int64)
        row = np.concatenate([row, selfn])
        col = np.concatenate([col, selfn])
        rows.append(row)
        slots.append((col - n0).astype(np.int32))
        norms.append((dinv[row] * dinv[col]).astype(np.float32))
        graphs.append(np.full(len(row), gi, np.int8))
    row = np.concatenate(rows)
    slot = np.concatenate(slots)
    norm = np.concatenate(norms)
    graph = np.concatenate(graphs)
    bank = slot >> 9
    grp = bank >> 1
    chunk = (row // CH).astype(np.int32)

    order = np.lexsort((slot, bank, graph, chunk, grp))
    row, slot, norm, graph, bank, grp, chunk = (
        a[order] for a in (row, slot, norm, graph, bank, grp, chunk))

    # cell = (grp, chunk, graph, bank); pad each to x128
    key = ((grp.astype(np.int64) * NCHUNK + chunk) * 2 + graph) * NBANK + bank
    uniq, starts = np.unique(key, return_index=True)
    starts = np.sort(starts)
    bounds = list(starts) + [len(key)]

    calls = []       # dicts: grp, chunk, idx (int16 local), windows list
    cur_call = None
    s_blocks = []    # per-window [128, B] fp32 one-hot*norm (quantized later)
    nwin = 0
    for s, e in zip(bounds[:-1], bounds[1:]):
        g_, b_, c_, gr_ = int(graph[s]), int(bank[s]), int(chunk[s]), int(grp[s])
        r_, sl_, nm_ = row[s:e], slot[s:e], norm[s:e]
        pad = (-len(r_)) % P
        if pad:
            r_ = np.concatenate([r_, np.full(pad, r_[-1], np.int64)])
            sl_ = np.concatenate([sl_, np.full(pad, sl_[-1], np.int32)])
            nm_ = np.concatenate([nm_, np.zeros(pad, np.float32)])
        if cur_call is None or cur_call["grp"] != gr_ or cur_call["chunk"] != c_:
            cur_call = {"grp": gr_, "chunk": c_, "idx": [], "windows": []}
            calls.append(cur_call)
        base = len(cur_call["idx"])
        cur_call["idx"].extend((r_ - c_ * CH).astype(np.int16))
        nw = len(r_) // P
        for w in range(nw):
            ssl = sl_[w * P:(w + 1) * P]
            snm = nm_[w * P:(w + 1) * P]
            smin = int(ssl.min())
            B = int(ssl.max()) - smin + 1
            S = np.zeros((P, B), np.float32)
            S[np.arange(P), ssl - smin] = snm
            cur_call["windows"].append({
                "wslot": base // P + w, "graph": g_, "bank": b_,
                "smin": smin - b_ * BANK, "B": B, "wid": nwin})
            s_blocks.append(S)
            nwin += 1
    # idx stream: per call, wrapped [16, n/16] replicated to 128 partitions
    itot = sum(len(c["idx"]) // 16 for c in calls)
    ia = np.zeros((P, itot), np.int16)
    off = 0
    for c in calls:
        idx = np.asarray(c["idx"], np.int16)
        wrapped = idx.reshape(-1, 16).T
        ia[:, off:off + wrapped.shape[1]] = np.tile(wrapped, (8, 1))
        c["i_col"] = off
        c["n"] = len(idx)
        off += wrapped.shape[1]
    return {"calls": calls, "ia": ia, "s_blocks": s_blocks, "NW": nwin}


# --------------------------- device programs ---------------------------

def build_run1():
    """h1 = x @ W1 for one shard; output fp8 (identical for all cores).
    Inputs: xT [512, SH] f16, w1 [512, 128] f16. Output: h [SH, 128] f8."""
    nc = bacc.Bacc(None, target_bir_lowering=False)
    xT = nc.dram_tensor("xT", [512, SH], F16, kind="ExternalInput")
    w1 = nc.dram_tensor("w1", [512, 128], F16, kind="ExternalInput")
    h = nc.dram_tensor("h", [SH, 128], F8, kind="ExternalOutput")
    KX = 4
    with tile.TileContext(nc) as tc:
        with (
            tc.tile_pool(name="const", bufs=1) as cp,
            tc.tile_pool(name="sb", bufs=3) as sb,
            tc.tile_pool(name="ev", bufs=3) as ev,
            tc.tile_pool(name="ps", bufs=4, space="PSUM") as ps,
        ):
            w1t = cp.tile([128, KX, 128], F16)
            for kc in range(KX):
                nc.sync.dma_start(out=w1t[:, kc, :], in_=w1[kc * 128:(kc + 1) * 128, :])
            xTfull = xT[:, :]
            hfull = h[:, :]
            for t in range(SH // 512):
                xt = sb.tile([128, KX, 512], F16, tag="xt")
                # one DMA: partition p, kc, col <- xT[kc*128+p, t*512+col]
                src = bass.AP(xTfull.tensor, t * 512,
                              [[SH, 128], [128 * SH, KX], [1, 512]])
                nc.sync.dma_start(out=xt[:], in_=src)
                he = ev.tile([128, 4, 128], F8, tag="he")
                for s in range(4):
                    pt = ps.tile([128, 128], F32, tag="h")
                    for kc in range(KX):
                        nc.tensor.matmul(
                            out=pt[:], lhsT=xt[:, kc, s * 128:(s + 1) * 128],
                            rhs=w1t[:, kc, :], start=(kc == 0), stop=(kc == KX - 1))
                    if s % 2 == 0:
                        nc.vector.tensor_copy(he[:, s, :], pt[:])
                    else:
                        nc.scalar.activation(he[:, s, :], pt[:],
                                             mybir.ActivationFunctionType.Copy)
                # one DMA: h row t*512 + s*128 + p
                dst = bass.AP(hfull.tensor, t * 512 * 128,
                              [[128, 128], [128 * 128, 4], [1, 128]])
                nc.sync.dma_start(out=dst, in_=he[:])
    nc.compile()
    return nc


def emit_agg_groups(nc, tc, pools, plan, table, elem, elem_step, tdtype,
                    ia, sa, consume_group):
    """Shared run2/run3 emit loop: per bank-group gather + aggregate, then
    call consume_group(grp, banks, psum_tiles) to drain."""
    cp, sb, sp, ps = pools
    calls_by_grp = {}
    for c in plan["calls"]:
        calls_by_grp.setdefault(c["grp"], []).append(c)
    for grp in range(NGRP):
        banks = [2 * grp] if 2 * grp == NBANK - 1 else [2 * grp, 2 * grp + 1]
        pt = {}
        for b in banks:
            for g in range(2):
                t_ = ps.tile([128, BANK], F32, tag=f"acc{g}{b & 1}")
                nc.vector.memset(t_[:], 0.0)
                pt[(g, b)] = t_
        slo, shi = plan["grp_s_range"][grp]
        st = None
        if shi > slo:
            st = sp.tile([128, shi - slo], F8, tag="st")
            nc.sync.dma_start(out=st[:], in_=sa[:, slo:shi])
        for call in calls_by_grp.get(grp, []):
            n = call["n"]
            it = sb.tile([128, n // 16], I16, tag="it")
            nc.sync.dma_start(out=it[:], in_=ia[:, call["i_col"]:call["i_col"] + n // 16])
            gt = sb.tile([128, n // P, elem], tdtype, tag="gt")
            c0 = call["chunk"] * CH
            raw_dma_gather(nc.gpsimd, gt[:],
                           table[c0:c0 + CH, 0:elem], it[:], n, elem, elem_step)
            for w in call["windows"]:
                B = w["B"]
                p_ = pt[(w["graph"], w["bank"])]
                nc.tensor.matmul(
                    out=p_[:elem, w["smin"]:w["smin"] + B],
                    lhsT=gt[:, w["wslot"], :],
                    rhs=st[:, w["s_col"]:w["s_col"] + B],
                    start=False, stop=True, skip_group_check=True)
        consume_group(grp, banks, pt)


def annotate_plan(plan):
    """Pack per-window S blocks into a per-group fp8 stream (4-col aligned
    slices) and record group S-column ranges."""
    import ml_dtypes
    grp_cols = {g: 0 for g in range(NGRP)}
    for c in plan["calls"]:
        for w in c["windows"]:
            w["s_col"] = grp_cols[c["grp"]]
            grp_cols[c["grp"]] += (w["B"] + 3) & ~3
    starts = {}
    off = 0
    for g in range(NGRP):
        starts[g] = off
        off += grp_cols[g]
    SBTOT = max(off, 4)
    s_arr = np.zeros((P, SBTOT), ml_dtypes.float8_e4m3)
    for c in plan["calls"]:
        for w in c["windows"]:
            S = plan["s_blocks"][w["wid"]]
            c0 = starts[c["grp"]] + w["s_col"]
            s_arr[:, c0:c0 + w["B"]] = S.astype(ml_dtypes.float8_e4m3)
    plan["s_arr"] = s_arr
    plan["grp_s_range"] = {g: (starts[g], starts[g] + grp_cols[g])
                           for g in range(NGRP)}
    del plan["s_blocks"]
    return plan


def build_run2(plan):
    """L1 aggregation (both graphs) + R1 + h2 = R1 @ W2 for one core."""
    nc = bacc.Bacc(None, target_bir_lowering=False)
    t1 = nc.dram_tensor("t1", [NPAD, 256], F8, kind="ExternalInput")
    ia = nc.dram_tensor("ia", [P, plan["ia"].shape[1]], I16, kind="ExternalInput")
    sa = nc.dram_tensor("sa", [P, plan["s_arr"].shape[1]], F8, kind="ExternalInput")
    w2 = nc.dram_tensor("w2", [256, 40], F16, kind="ExternalInput")
    b1v = nc.dram_tensor("b1v", [128, 1], F32, kind="ExternalInput")
    h2 = nc.dram_tensor("h2", [SH, 40], F8, kind="ExternalOutput")
    with tile.TileContext(nc) as tc:
        with (
            tc.tile_pool(name="const", bufs=1) as cp,
            tc.tile_pool(name="sb", bufs=3) as sb,
            tc.tile_pool(name="sp", bufs=4) as sp,
            tc.tile_pool(name="r1", bufs=2) as r1p,
            tc.tile_pool(name="ev", bufs=4) as ev,
            tc.tile_pool(name="ps", bufs=1, space="PSUM") as ps,
            tc.tile_pool(name="ps2", bufs=2, space="PSUM") as ps2,
        ):
            w2t = cp.tile([128, 2, 40], F16)
            for kc in range(2):
                nc.sync.dma_start(out=w2t[:, kc, :], in_=w2[kc * 128:(kc + 1) * 128, :])
            b1t = cp.tile([128, 1], F32)
            nc.sync.dma_start(out=b1t[:], in_=b1v[:])

            h2full = h2[:, :]

            def consume(grp, banks, pt):
                for b in banks:
                    r1a = r1p.tile([128, BANK], F16, tag="r1a")
                    r1b = r1p.tile([128, BANK], F16, tag="r1b")
                    nc.scalar.activation(r1a[:], pt[(0, b)][:],
                                         mybir.ActivationFunctionType.Relu,
                                         bias=b1t[:, :1], scale=1.0)
                    nc.scalar.activation(r1b[:], pt[(1, b)][:],
                                         mybir.ActivationFunctionType.Relu,
                                         bias=b1t[:, :1], scale=1.0)
                    p2 = ps2.tile([128, 4, 40], F32, tag="h2")
                    for s in range(BANK // P):
                        nc.tensor.matmul(out=p2[:, s, :], lhsT=r1a[:, s * P:(s + 1) * P],
                                         rhs=w2t[:, 0, :], start=True, stop=False)
                        nc.tensor.matmul(out=p2[:, s, :], lhsT=r1b[:, s * P:(s + 1) * P],
                                         rhs=w2t[:, 1, :], start=False, stop=True)
                    he = ev.tile([128, 4, 40], F8, tag="he")
                    if b % 2 == 0:
                        nc.vector.tensor_copy(he[:], p2[:])
                    else:
                        nc.scalar.activation(he[:], p2[:],
                                             mybir.ActivationFunctionType.Copy)
                    dst = bass.AP(h2full.tensor, b * BANK * 40,
                                  [[40, 128], [P * 40, 4], [1, 40]])
                    nc.sync.dma_start(out=dst, in_=he[:])

            emit_agg_groups(nc, tc, (cp, sb, sp, ps), plan, t1, 128, 256, F8,
                            ia, sa, consume)
    nc.compile()
    return nc


def build_run3(plan):
    """L2 aggregation (both graphs) + logits + log_softmax for one core."""
    nc = bacc.Bacc(None, target_bir_lowering=False)
    t2 = nc.dram_tensor("t2", [NPAD, 256], F8, kind="ExternalInput")
    ia = nc.dram_tensor("ia", [P, plan["ia"].shape[1]], I16, kind="ExternalInput")
    sa = nc.dram_tensor("sa", [P, plan["s_arr"].shape[1]], F8, kind="ExternalInput")
    wlt = nc.dram_tensor("wlt", [104, 40], F16, kind="ExternalInput")
    b2v = nc.dram_tensor("b2v", [128, 1], F32, kind="ExternalInput")
    blr = nc.dram_tensor("blr", [128, 40], F32, kind="ExternalInput")
    out = nc.dram_tensor("out", [SH, 40], F32, kind="ExternalOutput")
    with tile.TileContext(nc) as tc:
        with (
            tc.tile_pool(name="const", bufs=1) as cp,
            tc.tile_pool(name="sb", bufs=3) as sb,
            tc.tile_pool(name="sp", bufs=4) as sp,
            tc.tile_pool(name="r2", bufs=2) as r2p,
            tc.tile_pool(name="ev", bufs=6) as ev,
            tc.tile_pool(name="ps", bufs=1, space="PSUM") as ps,
            tc.tile_pool(name="ps2", bufs=2, space="PSUM") as ps2,
        ):
            wltt = cp.tile([104, 40], F16)
            nc.sync.dma_start(out=wltt[:], in_=wlt[:])
            b2t = cp.tile([128, 1], F32)
            nc.sync.dma_start(out=b2t[:], in_=b2v[:])
            blt = cp.tile([128, 1, 40], F32)
            nc.sync.dma_start(out=blt[:], in_=blr[:])

            outfull = out[:, :]

            def consume(grp, banks, pt):
                for b in banks:
                    r2t = r2p.tile([104, BANK], F16, tag="r2")
                    nc.vector.tensor_scalar_add(r2t[0:40, :], pt[(0, b)][:40, :],
                                                b2t[:40, :1])
                    nc.vector.tensor_scalar_add(r2t[64:104, :], pt[(1, b)][:40, :],
                                                b2t[:40, :1])
                    p2 = ps2.tile([128, 4, 40], F32, tag="lg")
                    for s in range(BANK // P):
                        nc.tensor.matmul(out=p2[:, s, :], lhsT=r2t[:, s * P:(s + 1) * P],
                                         rhs=wltt[:], start=True, stop=True)
                    lg = ev.tile([128, 4, 40], F32, tag="lg_sb")
                    nc.vector.tensor_tensor(out=lg[:], in0=p2[:],
                                            in1=blt[:].to_broadcast([128, 4, 40]),
                                            op=mybir.AluOpType.add)
                    mx = ev.tile([128, 4, 1], F32, tag="mx")
                    nc.vector.tensor_reduce(mx[:], lg[:], mybir.AxisListType.X,
                                            mybir.AluOpType.max)
                    xs = ev.tile([128, 4, 40], F32, tag="xs")
                    nc.vector.tensor_tensor(out=xs[:], in0=lg[:],
                                            in1=mx[:].to_broadcast([128, 4, 40]),
                                            op=mybir.AluOpType.subtract)
                    ex = ev.tile([128, 4, 40], F32, tag="ex")
                    nc.scalar.activation(ex[:], xs[:],
                                         mybir.ActivationFunctionType.Exp)
                    sm = ev.tile([128, 4, 1], F32, tag="sm")
                    nc.vector.tensor_reduce(sm[:], ex[:], mybir.AxisListType.X,
                                            mybir.AluOpType.add)
                    ls = ev.tile([128, 4, 1], F32, tag="ls")
                    nc.scalar.activation(ls[:], sm[:],
                                         mybir.ActivationFunctionType.Ln)
                    c_ = ev.tile([128, 4, 1], F32, tag="c")
                    nc.vector.tensor_add(c_[:], mx[:], ls[:])
                    fin = ev.tile([128, 4, 40], F32, tag="fin")
                    nc.vector.tensor_tensor(out=fin[:], in0=lg[:],
                                            in1=c_[:].to_broadcast([128, 4, 40]),
                                            op=mybir.AluOpType.subtract)
                    dst = bass.AP(outfull.tensor, b * BANK * 40,
                                  [[40, 128], [P * 40, 4], [1, 40]])
                    nc.sync.dma_start(out=dst, in_=fin[:])

            emit_agg_groups(nc, tc, (cp, sb, sp, ps), plan, t2, 40, 256, F8,
                            ia, sa, consume)
    nc.compile()
    return nc


# --------------------------- execution harness ---------------------------

class DeviceProgram:
    def __init__(self, nc, device):
        install_neuronx_cc_hook()
        self.nc = nc
        self.device = device
        partition_name = nc.partition_id_tensor.name if nc.partition_id_tensor else None
        in_names, out_names, out_avals, zero_outs = [], [], [], []
        for alloc in nc.m.functions[0].allocations:
            if not isinstance(alloc, mybir.MemoryLocationSet):
                continue
            name = alloc.memorylocations[0].name
            if alloc.kind == "ExternalInput":
                if name != partition_name:
                    in_names.append(name)
            elif alloc.kind == "ExternalOutput":
                shape = tuple(alloc.tensor_shape)
                dtype = mybir.dt.np(alloc.dtype)
                out_names.append(name)
                out_avals.append(jax.core.ShapedArray(shape, dtype))
                zero_outs.append(np.zeros(shape, dtype))
        self.in_names = list(in_names)
        self.out_names = out_names
        self.out_avals = out_avals
        self.zero_outs = zero_outs
        n_params = len(in_names)
        all_names = in_names + out_names + ([partition_name] if partition_name else [])
        self.n_params = n_params
        donate = tuple(range(n_params, n_params + len(out_names)))

        def _body(*args):
            operands = list(args)
            if partition_name is not None:
                operands.append(partition_id_tensor())
            outs = _bass_exec_p.bind(
                *operands,
                out_avals=tuple(out_avals),
                in_names=tuple(all_names),
                out_names=tuple(out_names),
                lowering_input_output_aliases=(),
                sim_require_finite=True,
                sim_require_nnan=True,
                nc=nc,
            )
            return tuple(outs)

        self.fn = jax.jit(_body, donate_argnums=donate, keep_unused=True)
        self.dev_inputs = None

    def upload(self, in_map):
        arrs = [np.asarray(in_map[n]) for n in self.in_names]
        self.dev_inputs = [jax.device_put(a, self.device) for a in arrs]

    def call(self):
        zo = [jax.device_put(z, self.device) for z in self.zero_outs]
        outs = self.fn(*self.dev_inputs, *zo)
        return outs

    def results(self, outs):
        return {n: np.asarray(o) for n, o in zip(self.out_names, outs)}


def _parallel(fns):
    outs = [None] * len(fns)
    errs = []

    def wrap(i):
        try:
            outs[i] = fns[i]()
        except Exception as e:  # noqa: BLE001
            import traceback
            errs.append((i, e, traceback.format_exc()))

    ts = [threading.Thread(target=wrap, args=(i,)) for i in range(len(fns))]
    for t in ts:
        t.start()
    for t in ts:
        t.join()
    if errs:
        raise RuntimeError(f"thread errors: {[(i, tb) for i, _, tb in errs]}")
    return outs


class Pipeline:
    def __init__(self, inputs, verbose=True):
        self.v = verbose
        self.inputs = inputs
        self.devices = jax.devices()[:N_CORES]
        t0 = time.time()
        self.plans = [annotate_plan(build_plan(
            inputs["edge_index"], inputs["edge_index_knn"], c))
            for c in range(N_CORES)]
        if verbose:
            print(f"[prep] plans {time.time()-t0:.1f}s", flush=True)
        t0 = time.time()
        nc1 = build_run1()
        if verbose:
            print(f"[build] run1 {time.time()-t0:.1f}s", flush=True)
        nc2s, nc3s = [], []
        for core in range(N_CORES):
            t = time.time()
            nc2s.append(build_run2(self.plans[core]))
            nc3s.append(build_run3(self.plans[core]))
            if verbose:
                print(f"[build] core {core} run2+run3 {time.time()-t:.1f}s", flush=True)
        t0 = time.time()
        self.p1 = [DeviceProgram(nc1, self.devices[i]) for i in range(N_CORES)]
        self.p2 = [DeviceProgram(nc2s[i], self.devices[i]) for i in range(N_CORES)]
        self.p3 = [DeviceProgram(nc3s[i], self.devices[i]) for i in range(N_CORES)]
        if verbose:
            print(f"[build] DevicePrograms {time.time()-t0:.1f}s", flush=True)
        self._prepare_inputs()

    def _prepare_inputs(self):
        ins = self.inputs
        x = np.asarray(ins["x"])
        W1 = np.asarray(ins["W1"]).astype(np.float16)
        W2 = np.asarray(ins["W2"]).astype(np.float16)
        Wlin = np.asarray(ins["Wlin"]).astype(np.float16)
        b1 = np.asarray(ins["b1"]).astype(np.float32)
        b2 = np.asarray(ins["b2"]).astype(np.float32)
        blin = np.asarray(ins["blin"]).astype(np.float32)

        w1p = np.zeros((512, 128), np.float16)
        w1p[:500] = W1
        b1v = b1[:, None]
        b2v = np.zeros((128, 1), np.float32)
        b2v[:40, 0] = b2
        blr = np.tile(blin[None, :], (128, 1)).astype(np.float32)
        wlt = np.zeros((104, 40), np.float16)
        wlt[0:40] = Wlin.T[0:40]
        wlt[64:104] = Wlin.T[40:80]

        self.run1_maps = []
        for i in range(N_CORES):
            xs = np.zeros((SH, 512), np.float16)
            lo, hi = i * SH, min((i + 1) * SH, N_REAL)
            if hi > lo:
                xs[:hi - lo, :500] = x[lo:hi].astype(np.float16)
            self.run1_maps.append({"xT": np.ascontiguousarray(xs.T), "w1": w1p})
        self.consts2 = {"w2": W2, "b1v": b1v}
        self.consts3 = {"wlt": wlt, "b2v": b2v, "blr": blr}

    def run(self, time_it=False):
        v = self.v
        t0 = time.time()
        for i in range(N_CORES):
            self.p1[i].upload(self.run1_maps[i])
        outs1 = _parallel([self.p1[i].call for i in range(N_CORES)])
        h_shards = [self.p1[i].results(outs1[i])["h"] for i in range(N_CORES)]
        table1 = np.zeros((NPAD, 256), h_shards[0].dtype)
        table1[:, :128] = np.concatenate(h_shards, axis=0)
        if v:
            print(f"[run1] done {time.time()-t0:.1f}s", flush=True)

        t0 = time.time()
        for i in range(N_CORES):
            pl = self.plans[i]
            self.p2[i].upload({"t1": table1, "ia": pl["ia"], "sa": pl["s_arr"],
                               **self.consts2})
        outs2 = _parallel([self.p2[i].call for i in range(N_CORES)])
        h2_shards = [self.p2[i].results(outs2[i])["h2"] for i in range(N_CORES)]
        table2 = np.zeros((NPAD, 256), h2_shards[0].dtype)
        table2[:, :40] = np.concatenate(h2_shards, axis=0)
        if v:
            print(f"[run2] done {time.time()-t0:.1f}s", flush=True)

        t0 = time.time()
        for i in range(N_CORES):
            pl = self.plans[i]
            self.p3[i].upload({"t2": table2, "ia": pl["ia"], "sa": pl["s_arr"],
                               **self.consts3})
        outs3 = _parallel([self.p3[i].call for i in range(N_CORES)])
        out_shards = [self.p3[i].results(outs3[i])["out"] for i in range(N_CORES)]
        result = np.concatenate(out_shards, axis=0)[:N_REAL]
        if v:
            print(f"[run3] done {time.time()-t0:.1f}s", flush=True)

        times = None
        if time_it:
            times = self.time_runs()
        return result, times

    def time_runs(self, reps=5):
        times = {}
        for name, progs in (("run1", self.p1), ("run2", self.p2), ("run3", self.p3)):
            best = float("inf")
            for _ in range(reps):
                barrier = threading.Barrier(N_CORES + 1)
                done = []

                def worker(p):
                    barrier.wait()
                    o = p.call()
                    jax.block_until_ready(o)
                    done.append(o)

                ts = [threading.Thread(target=worker, args=(p,)) for p in progs]
                for t in ts:
                    t.start()
                barrier.wait()
                t0 = time.time()
                for t in ts:
                    t.join()
                best = min(best, time.time() - t0)
            times[name] = best
        return times


_PIPELINE_CACHE = {}


def kernel(**inputs):
    key = "singleton"
    pl = _PIPELINE_CACHE.get(key)
    if pl is None or pl.graph_key != _graph_key(inputs):
        pl = Pipeline(inputs, verbose=False)
        pl.graph_key = _graph_key(inputs)
        _PIPELINE_CACHE[key] = pl
    else:
        pl.inputs = inputs
        pl._prepare_inputs()
    out, _ = pl.run(time_it=False)
    return out.astype(np.float32)


def _graph_key(inputs):
    ei = np.asarray(inputs["edge_index"])
    ek = np.asarray(inputs["edge_index_knn"])
    return (ei.shape, ek.shape, int(ei[:, 0].sum()), int(ei[:, -1].sum()),
            int(ek[:, 0].sum()), int(ek[:, -1].sum()))


# revision 4
# speedup vs baseline: 1.3342x; 1.0502x over previous
"""Self-contained Trainium2 Bass kernel for the 2-layer dual-graph GCN
(nn_GCN0100). Accepts FULL inputs, returns FULL output.

Strategy: node-sharded across 8 NeuronCores, 3 SPMD-style launches:
  run1: h1 = x @ W1 per shard, emitted fp8 (table1)
  run2: layer-1 gather/segment-sum over both graphs (dma_gather of fp8
        128-B rows + on-device one-hot S build + matmul into PSUM),
        ReLU+bias, h2 = R1 @ W2 -> fp16 table2 shard
  run3: layer-2 gather/segment-sum (80-B partial-row fp16 gathers),
        logits, log_softmax
Host assembles the full tables between launches (free halo exchange) and
does index-only graph preprocessing; all FLOPs / per-edge data movement
run on device.
"""
import threading
import time
import numpy as np
import jax
import concourse.bass as bass
import concourse.mybir as mybir
import concourse.tile as tile
import concourse.ap_utils as ap_utils
from concourse import bacc
from concourse.bass2jax import _bass_exec_p, partition_id_tensor, install_neuronx_cc_hook


P = 128
SH = 12800          # shard size (102400 / 8)
NPAD = 102400       # padded node count
CH = 25600          # gather chunk rows (int16 idx limit)
NCHUNK = NPAD // CH  # 4
BANK = 512          # PSUM bank slots
NBANK = SH // BANK   # 25
NGRP = (NBANK + 1) // 2  # 13 bank groups of 2 (last has 1)
N_CORES = 8
N_REAL = 100000

F8 = mybir.dt.float8e4
F16 = mybir.dt.float16
F32 = mybir.dt.float32
I16 = mybir.dt.int16


def raw_dma_gather(g, out_ap, in_ap, idxs_ap, num_idxs, elem_size, elem_step):
    """dma_gather with elem_size < row stride (partial-row reads).

    Same encoding as BassGpSimd.dma_gather (non-transpose, DRAM source)
    minus the elem_size%256 assert: the real constraint is that the row
    STRIDE is a multiple of 256B; the read size per descriptor may be
    smaller (verified on hardware)."""
    assert idxs_ap.dtype == mybir.dt.int16
    assert in_ap.dtype == out_ap.dtype
    assert ap_utils.ap_is_contiguous(in_ap.ap[1:])
    assert ap_utils.ap_is_contiguous(out_ap.ap[1:])
    assert ap_utils.ap_is_contiguous(idxs_ap.ap[1:])
    assert in_ap.ap[-1][1] == out_ap.ap[-1][1] == elem_size
    assert in_ap.ap[0][0] == elem_step
    stride_bytes = elem_step * mybir.dt.size(in_ap.dtype)
    assert stride_bytes % 256 == 0 and stride_bytes // 256 < 256
    _in_ap = g.lower_ap_dma(in_ap, for_custom_bir_dma=True)
    _idxs_ap = g.lower_ap(idxs_ap)
    _out_ap = g.lower_ap(out_ap)
    return g.add_instruction(mybir.InstDMAGatherAnt(
        name=g.bass.get_next_instruction_name(),
        ins=[*_in_ap, _idxs_ap, g.lower_val_access(g.to_reg(num_idxs))],
        outs=[_out_ap], transpose=False, num_idxs=num_idxs,
        elem_size=elem_size, stride_bytes_256=stride_bytes // 256,
        gen_mode=0, single_packet=False, queue_num=0,
        sbuf_tokens_per_rank=0, sbuf_free_dim_per_rank=0,
        sbuf_free_dim_pad_per_rank=0, sbuf_byte_offset=0))


def degrees_dinv(edge_index, n=N_REAL):
    deg = np.bincount(np.asarray(edge_index[1]), minlength=n).astype(np.float64) + 1.0
    return (1.0 / np.sqrt(deg)).astype(np.float32)


def build_plan(edge_index, edge_index_knn, core):
    """One merged plan per core, shared by run2 and run3 (same edges).

    Returns dict with device streams (ia / ws / wn) and call/window
    metadata for the emit loop."""
    dinv_s = degrees_dinv(edge_index)
    dinv_k = degrees_dinv(edge_index_knn)
    n0, n1 = core * SH, core * SH + SH
    rows, slots, norms, graphs = [], [], [], []
    for gi, (ei, dinv) in enumerate(((edge_index, dinv_s), (edge_index_knn, dinv_k))):
        row = np.asarray(ei[0]).astype(np.int64)
        col = np.asarray(ei[1]).astype(np.int64)
        m = (col >= n0) & (col < n1)
        row, col = row[m], col[m]
        selfn = np.arange(n0, min(n1, N_REAL), dtype=np.int64)
        row = np.concatenate([row, selfn])
        col = np.concatenate([col, selfn])
        rows.append(row)
        slots.append((col - n0).astype(np.int32))
        norms.append((dinv[row] * dinv[col]).astype(np.float32))
        graphs.append(np.full(len(row), gi, np.int8))
    row = np.concatenate(rows)
    slot = np.concatenate(slots)
    norm = np.concatenate(norms)
    graph = np.concatenate(graphs)
    bank = slot >> 9
    grp = bank >> 1
    chunk = (row // CH).astype(np.int32)

    order = np.lexsort((slot, bank, graph, chunk, grp))
    row, slot, norm, graph, bank, grp, chunk = (
        a[order] for a in (row, slot, norm, graph, bank, grp, chunk))

    # cell = (grp, chunk, graph, bank); pad each to x128
    key = ((grp.astype(np.int64) * NCHUNK + chunk) * 2 + graph) * NBANK + bank
    uniq, starts = np.unique(key, return_index=True)
    starts = np.sort(starts)
    bounds = list(starts) + [len(key)]

    calls = []       # dicts: grp, chunk, idx (int16 local), windows list
    cur_call = None
    s_blocks = []    # per-window [128, B] fp32 one-hot*norm (quantized later)
    nwin = 0
    for s, e in zip(bounds[:-1], bounds[1:]):
        g_, b_, c_, gr_ = int(graph[s]), int(bank[s]), int(chunk[s]), int(grp[s])
        r_, sl_, nm_ = row[s:e], slot[s:e], norm[s:e]
        pad = (-len(r_)) % P
        if pad:
            r_ = np.concatenate([r_, np.full(pad, r_[-1], np.int64)])
            sl_ = np.concatenate([sl_, np.full(pad, sl_[-1], np.int32)])
            nm_ = np.concatenate([nm_, np.zeros(pad, np.float32)])
        if cur_call is None or cur_call["grp"] != gr_ or cur_call["chunk"] != c_:
            cur_call = {"grp": gr_, "chunk": c_, "idx": [], "windows": []}
            calls.append(cur_call)
        base = len(cur_call["idx"])
        cur_call["idx"].extend((r_ - c_ * CH).astype(np.int16))
        nw = len(r_) // P
        for w in range(nw):
            ssl = sl_[w * P:(w + 1) * P]
            snm = nm_[w * P:(w + 1) * P]
            smin = int(ssl.min())
            B = int(ssl.max()) - smin + 1
            S = np.zeros((P, B), np.float32)
            S[np.arange(P), ssl - smin] = snm
            cur_call["windows"].append({
                "wslot": base // P + w, "graph": g_, "bank": b_,
                "smin": smin - b_ * BANK, "B": B, "wid": nwin})
            s_blocks.append(S)
            nwin += 1
    # idx stream: per call, wrapped [16, n/16] replicated to 128 partitions
    itot = sum(len(c["idx"]) // 16 for c in calls)
    ia = np.zeros((P, itot), np.int16)
    off = 0
    for c in calls:
        idx = np.asarray(c["idx"], np.int16)
        wrapped = idx.reshape(-1, 16).T
        ia[:, off:off + wrapped.shape[1]] = np.tile(wrapped, (8, 1))
        c["i_col"] = off
        c["n"] = len(idx)
        off += wrapped.shape[1]
    return {"calls": calls, "ia": ia, "s_blocks": s_blocks, "NW": nwin}


# --------------------------- device programs ---------------------------

def build_run1():
    """h1 = x @ W1 for one shard; output fp8 (identical for all cores).
    Inputs: xT [512, SH] f16, w1 [512, 128] f16. Output: h [SH, 128] f8."""
    nc = bacc.Bacc(None, target_bir_lowering=False)
    xT = nc.dram_tensor("xT", [512, SH], F8, kind="ExternalInput")
    w1 = nc.dram_tensor("w1", [512, 128], F16, kind="ExternalInput")
    h = nc.dram_tensor("h", [SH, 128], F8, kind="ExternalOutput")
    KX = 4
    with tile.TileContext(nc) as tc:
        with (
            tc.tile_pool(name="const", bufs=1) as cp,
            tc.tile_pool(name="sb", bufs=3) as sb,
            tc.tile_pool(name="ev", bufs=3) as ev,
            tc.tile_pool(name="ps", bufs=4, space="PSUM") as ps,
        ):
            w1t = cp.tile([128, KX, 128], F16)
            for kc in range(KX):
                nc.sync.dma_start(out=w1t[:, kc, :], in_=w1[kc * 128:(kc + 1) * 128, :])
            xTfull = xT[:, :]
            hfull = h[:, :]
            for t in range(SH // 512):
                xt = sb.tile([128, KX, 512], F8, tag="xt")
                # one DMA: partition p, kc, col <- xT[kc*128+p, t*512+col]
                src = bass.AP(xTfull.tensor, t * 512,
                              [[SH, 128], [128 * SH, KX], [1, 512]])
                nc.sync.dma_start(out=xt[:], in_=src)
                he = ev.tile([128, 4, 128], F8, tag="he")
                for s in range(4):
                    pt = ps.tile([128, 128], F32, tag="h")
                    for kc in range(KX):
                        nc.tensor.matmul(
                            out=pt[:], lhsT=xt[:, kc, s * 128:(s + 1) * 128],
                            rhs=w1t[:, kc, :], start=(kc == 0), stop=(kc == KX - 1))
                    if s % 2 == 0:
                        nc.vector.tensor_copy(he[:, s, :], pt[:])
                    else:
                        nc.scalar.activation(he[:, s, :], pt[:],
                                             mybir.ActivationFunctionType.Copy)
                # one DMA: h row t*512 + s*128 + p
                dst = bass.AP(hfull.tensor, t * 512 * 128,
                              [[128, 128], [128 * 128, 4], [1, 128]])
                nc.sync.dma_start(out=dst, in_=he[:])
    nc.compile()
    return nc


def emit_agg_groups(nc, tc, pools, plan, table, elem, elem_step, tdtype,
                    ia, sa, consume_group):
    """Shared run2/run3 emit loop: per bank-group gather + aggregate, then
    call consume_group(grp, banks, psum_tiles) to drain."""
    cp, sb, sp, ps = pools
    calls_by_grp = {}
    for c in plan["calls"]:
        calls_by_grp.setdefault(c["grp"], []).append(c)
    for grp in range(NGRP):
        banks = [2 * grp] if 2 * grp == NBANK - 1 else [2 * grp, 2 * grp + 1]
        pt = {}
        for b in banks:
            for g in range(2):
                t_ = ps.tile([128, BANK], F32, tag=f"acc{g}{b & 1}")
                if (g, b) not in plan["nonempty"]:
                    nc.vector.memset(t_[:], 0.0)
                pt[(g, b)] = t_
        slo, shi = plan["grp_s_range"][grp]
        st = None
        if shi > slo:
            st = sp.tile([128, shi - slo], F8, tag="st")
            nc.sync.dma_start(out=st[:], in_=sa[:, slo:shi])
        for call in calls_by_grp.get(grp, []):
            n = call["n"]
            it = sb.tile([128, n // 16], I16, tag="it")
            nc.sync.dma_start(out=it[:], in_=ia[:, call["i_col"]:call["i_col"] + n // 16])
            gt = sb.tile([128, n // P, elem], tdtype, tag="gt")
            c0 = call["chunk"] * CH
            raw_dma_gather(nc.gpsimd, gt[:],
                           table[c0:c0 + CH, 0:elem], it[:], n, elem, elem_step)
            for w in call["windows"]:
                B = w["B"]
                p_ = pt[(w["graph"], w["bank"])]
                nc.tensor.matmul(
                    out=p_[:elem, w["smin"]:w["smin"] + B],
                    lhsT=gt[:, w["wslot"], :],
                    rhs=st[:, w["s_col"]:w["s_col"] + B],
                    start=w["start"], stop=True, skip_group_check=True)
        consume_group(grp, banks, pt)


def annotate_plan(plan):
    """Pack per-window S blocks into a per-group fp8 stream (4-col aligned
    slices) and record group S-column ranges."""
    import ml_dtypes
    # first window of each (graph, bank) becomes a full-bank start=True
    # matmul (replaces the PSUM memset)
    seen = set()
    for c in plan["calls"]:
        for w in c["windows"]:
            key = (w["graph"], w["bank"])
            if key not in seen:
                seen.add(key)
                S = plan["s_blocks"][w["wid"]]
                full = np.zeros((P, BANK), np.float32)
                full[:, w["smin"]:w["smin"] + w["B"]] = S
                plan["s_blocks"][w["wid"]] = full
                w["smin"], w["B"], w["start"] = 0, BANK, True
            else:
                w["start"] = False
    plan["nonempty"] = seen
    grp_cols = {g: 0 for g in range(NGRP)}
    for c in plan["calls"]:
        for w in c["windows"]:
            w["s_col"] = grp_cols[c["grp"]]
            grp_cols[c["grp"]] += (w["B"] + 3) & ~3
    starts = {}
    off = 0
    for g in range(NGRP):
        starts[g] = off
        off += grp_cols[g]
    SBTOT = max(off, 4)
    s_arr = np.zeros((P, SBTOT), ml_dtypes.float8_e4m3)
    for c in plan["calls"]:
        for w in c["windows"]:
            S = plan["s_blocks"][w["wid"]]
            c0 = starts[c["grp"]] + w["s_col"]
            s_arr[:, c0:c0 + w["B"]] = S.astype(ml_dtypes.float8_e4m3)
    plan["s_arr"] = s_arr
    plan["grp_s_range"] = {g: (starts[g], starts[g] + grp_cols[g])
                           for g in range(NGRP)}
    del plan["s_blocks"]
    return plan


def build_run2(plan):
    """L1 aggregation (both graphs) + R1 + h2 = R1 @ W2 for one core."""
    nc = bacc.Bacc(None, target_bir_lowering=False)
    t1 = nc.dram_tensor("t1", [NPAD, 256], F8, kind="ExternalInput")
    ia = nc.dram_tensor("ia", [P, plan["ia"].shape[1]], I16, kind="ExternalInput")
    sa = nc.dram_tensor("sa", [P, plan["s_arr"].shape[1]], F8, kind="ExternalInput")
    w2 = nc.dram_tensor("w2", [256, 40], F16, kind="ExternalInput")
    b1v = nc.dram_tensor("b1v", [128, 1], F32, kind="ExternalInput")
    h2 = nc.dram_tensor("h2", [SH, 40], F8, kind="ExternalOutput")
    with tile.TileContext(nc) as tc:
        with (
            tc.tile_pool(name="const", bufs=1) as cp,
            tc.tile_pool(name="sb", bufs=3) as sb,
            tc.tile_pool(name="sp", bufs=4) as sp,
            tc.tile_pool(name="r1", bufs=2) as r1p,
            tc.tile_pool(name="ev", bufs=4) as ev,
            tc.tile_pool(name="ps", bufs=1, space="PSUM") as ps,
            tc.tile_pool(name="ps2", bufs=2, space="PSUM") as ps2,
        ):
            w2t = cp.tile([128, 2, 40], F16)
            for kc in range(2):
                nc.sync.dma_start(out=w2t[:, kc, :], in_=w2[kc * 128:(kc + 1) * 128, :])
            b1t = cp.tile([128, 1], F32)
            nc.sync.dma_start(out=b1t[:], in_=b1v[:])

            h2full = h2[:, :]

            def consume(grp, banks, pt):
                for b in banks:
                    r1a = r1p.tile([128, BANK], F16, tag="r1a")
                    r1b = r1p.tile([128, BANK], F16, tag="r1b")
                    nc.scalar.activation(r1a[:], pt[(0, b)][:],
                                         mybir.ActivationFunctionType.Relu,
                                         bias=b1t[:, :1], scale=1.0)
                    nc.scalar.activation(r1b[:], pt[(1, b)][:],
                                         mybir.ActivationFunctionType.Relu,
                                         bias=b1t[:, :1], scale=1.0)
                    p2 = ps2.tile([128, 4, 40], F32, tag="h2")
                    for s in range(BANK // P):
                        nc.tensor.matmul(out=p2[:, s, :], lhsT=r1a[:, s * P:(s + 1) * P],
                                         rhs=w2t[:, 0, :], start=True, stop=False)
                        nc.tensor.matmul(out=p2[:, s, :], lhsT=r1b[:, s * P:(s + 1) * P],
                                         rhs=w2t[:, 1, :], start=False, stop=True)
                    he = ev.tile([128, 4, 40], F8, tag="he")
                    if b % 2 == 0:
                        nc.vector.tensor_copy(he[:], p2[:])
                    else:
                        nc.scalar.activation(he[:], p2[:],
                                             mybir.ActivationFunctionType.Copy)
                    dst = bass.AP(h2full.tensor, b * BANK * 40,
                                  [[40, 128], [P * 40, 4], [1, 40]])
                    nc.sync.dma_start(out=dst, in_=he[:])

            emit_agg_groups(nc, tc, (cp, sb, sp, ps), plan, t1, 128, 256, F8,
                            ia, sa, consume)
    nc.compile()
    return nc


def build_run3(plan):
    """L2 aggregation (both graphs) + logits + log_softmax for one core."""
    nc = bacc.Bacc(None, target_bir_lowering=False)
    t2 = nc.dram_tensor("t2", [NPAD, 256], F8, kind="ExternalInput")
    ia = nc.dram_tensor("ia", [P, plan["ia"].shape[1]], I16, kind="ExternalInput")
    sa = nc.dram_tensor("sa", [P, plan["s_arr"].shape[1]], F8, kind="ExternalInput")
    wlt = nc.dram_tensor("wlt", [104, 40], F16, kind="ExternalInput")
    b2v = nc.dram_tensor("b2v", [128, 1], F32, kind="ExternalInput")
    blr = nc.dram_tensor("blr", [128, 40], F32, kind="ExternalInput")
    out = nc.dram_tensor("out", [SH, 40], F32, kind="ExternalOutput")
    with tile.TileContext(nc) as tc:
        with (
            tc.tile_pool(name="const", bufs=1) as cp,
            tc.tile_pool(name="sb", bufs=3) as sb,
            tc.tile_pool(name="sp", bufs=4) as sp,
            tc.tile_pool(name="r2", bufs=2) as r2p,
            tc.tile_pool(name="ev", bufs=6) as ev,
            tc.tile_pool(name="ps", bufs=1, space="PSUM") as ps,
            tc.tile_pool(name="ps2", bufs=2, space="PSUM") as ps2,
        ):
            wltt = cp.tile([104, 40], F16)
            nc.sync.dma_start(out=wltt[:], in_=wlt[:])
            b2t = cp.tile([128, 1], F32)
            nc.sync.dma_start(out=b2t[:], in_=b2v[:])
            blt = cp.tile([128, 1, 40], F32)
            nc.sync.dma_start(out=blt[:], in_=blr[:])

            outfull = out[:, :]

            def consume(grp, banks, pt):
                for b in banks:
                    r2t = r2p.tile([104, BANK], F16, tag="r2")
                    nc.vector.tensor_scalar_add(r2t[0:40, :], pt[(0, b)][:40, :],
                                                b2t[:40, :1])
                    nc.vector.tensor_scalar_add(r2t[64:104, :], pt[(1, b)][:40, :],
                                                b2t[:40, :1])
                    p2 = ps2.tile([128, 4, 40], F32, tag="lg")
                    for s in range(BANK // P):
                        nc.tensor.matmul(out=p2[:, s, :], lhsT=r2t[:, s * P:(s + 1) * P],
                                         rhs=wltt[:], start=True, stop=True)
                    lg = ev.tile([128, 4, 40], F32, tag="lg_sb")
                    nc.vector.tensor_tensor(out=lg[:], in0=p2[:],
                                            in1=blt[:].to_broadcast([128, 4, 40]),
                                            op=mybir.AluOpType.add)
                    mx = ev.tile([128, 4, 1], F32, tag="mx")
                    nc.vector.tensor_reduce(mx[:], lg[:], mybir.AxisListType.X,
                                            mybir.AluOpType.max)
                    xs = ev.tile([128, 4, 40], F32, tag="xs")
                    nc.vector.tensor_tensor(out=xs[:], in0=lg[:],
                                            in1=mx[:].to_broadcast([128, 4, 40]),
                                            op=mybir.AluOpType.subtract)
                    ex = ev.tile([128, 4, 40], F32, tag="ex")
                    nc.scalar.activation(ex[:], xs[:],
                                         mybir.ActivationFunctionType.Exp)
                    sm = ev.tile([128, 4, 1], F32, tag="sm")
                    nc.vector.tensor_reduce(sm[:], ex[:], mybir.AxisListType.X,
                                            mybir.AluOpType.add)
                    ls = ev.tile([128, 4, 1], F32, tag="ls")
                    nc.scalar.activation(ls[:], sm[:],
                                         mybir.ActivationFunctionType.Ln)
                    c_ = ev.tile([128, 4, 1], F32, tag="c")
                    nc.vector.tensor_add(c_[:], mx[:], ls[:])
                    fin = ev.tile([128, 4, 40], F32, tag="fin")
                    nc.vector.tensor_tensor(out=fin[:], in0=lg[:],
                                            in1=c_[:].to_broadcast([128, 4, 40]),
                                            op=mybir.AluOpType.subtract)
                    dst = bass.AP(outfull.tensor, b * BANK * 40,
                                  [[40, 128], [P * 40, 4], [1, 40]])
                    nc.sync.dma_start(out=dst, in_=fin[:])

            emit_agg_groups(nc, tc, (cp, sb, sp, ps), plan, t2, 40, 256, F8,
                            ia, sa, consume)
    nc.compile()
    return nc


# --------------------------- execution harness ---------------------------

class DeviceProgram:
    def __init__(self, nc, device):
        install_neuronx_cc_hook()
        self.nc = nc
        self.device = device
        partition_name = nc.partition_id_tensor.name if nc.partition_id_tensor else None
        in_names, out_names, out_avals, zero_outs = [], [], [], []
        for alloc in nc.m.functions[0].allocations:
            if not isinstance(alloc, mybir.MemoryLocationSet):
                continue
            name = alloc.memorylocations[0].name
            if alloc.kind == "ExternalInput":
                if name != partition_name:
                    in_names.append(name)
            elif alloc.kind == "ExternalOutput":
                shape = tuple(alloc.tensor_shape)
                dtype = mybir.dt.np(alloc.dtype)
                out_names.append(name)
                out_avals.append(jax.core.ShapedArray(shape, dtype))
                zero_outs.append(np.zeros(shape, dtype))
        self.in_names = list(in_names)
        self.out_names = out_names
        self.out_avals = out_avals
        self.zero_outs = zero_outs
        n_params = len(in_names)
        all_names = in_names + out_names + ([partition_name] if partition_name else [])
        self.n_params = n_params
        donate = tuple(range(n_params, n_params + len(out_names)))

        def _body(*args):
            operands = list(args)
            if partition_name is not None:
                operands.append(partition_id_tensor())
            outs = _bass_exec_p.bind(
                *operands,
                out_avals=tuple(out_avals),
                in_names=tuple(all_names),
                out_names=tuple(out_names),
                lowering_input_output_aliases=(),
                sim_require_finite=True,
                sim_require_nnan=True,
                nc=nc,
            )
            return tuple(outs)

        self.fn = jax.jit(_body, donate_argnums=donate, keep_unused=True)
        self.dev_inputs = None

    def upload(self, in_map):
        arrs = [np.asarray(in_map[n]) for n in self.in_names]
        self.dev_inputs = [jax.device_put(a, self.device) for a in arrs]

    def call(self):
        zo = [jax.device_put(z, self.device) for z in self.zero_outs]
        outs = self.fn(*self.dev_inputs, *zo)
        return outs

    def results(self, outs):
        return {n: np.asarray(o) for n, o in zip(self.out_names, outs)}


def _parallel(fns):
    outs = [None] * len(fns)
    errs = []

    def wrap(i):
        try:
            outs[i] = fns[i]()
        except Exception as e:  # noqa: BLE001
            import traceback
            errs.append((i, e, traceback.format_exc()))

    ts = [threading.Thread(target=wrap, args=(i,)) for i in range(len(fns))]
    for t in ts:
        t.start()
    for t in ts:
        t.join()
    if errs:
        raise RuntimeError(f"thread errors: {[(i, tb) for i, _, tb in errs]}")
    return outs


class Pipeline:
    def __init__(self, inputs, verbose=True):
        self.v = verbose
        self.inputs = inputs
        self.devices = jax.devices()[:N_CORES]
        t0 = time.time()
        self.plans = [annotate_plan(build_plan(
            inputs["edge_index"], inputs["edge_index_knn"], c))
            for c in range(N_CORES)]
        if verbose:
            print(f"[prep] plans {time.time()-t0:.1f}s", flush=True)
        t0 = time.time()
        nc1 = build_run1()
        if verbose:
            print(f"[build] run1 {time.time()-t0:.1f}s", flush=True)
        nc2s, nc3s = [], []
        for core in range(N_CORES):
            t = time.time()
            nc2s.append(build_run2(self.plans[core]))
            nc3s.append(build_run3(self.plans[core]))
            if verbose:
                print(f"[build] core {core} run2+run3 {time.time()-t:.1f}s", flush=True)
        t0 = time.time()
        self.p1 = [DeviceProgram(nc1, self.devices[i]) for i in range(N_CORES)]
        self.p2 = [DeviceProgram(nc2s[i], self.devices[i]) for i in range(N_CORES)]
        self.p3 = [DeviceProgram(nc3s[i], self.devices[i]) for i in range(N_CORES)]
        if verbose:
            print(f"[build] DevicePrograms {time.time()-t0:.1f}s", flush=True)
        self._prepare_inputs()

    def _prepare_inputs(self):
        ins = self.inputs
        x = np.asarray(ins["x"])
        W1 = np.asarray(ins["W1"]).astype(np.float16)
        W2 = np.asarray(ins["W2"]).astype(np.float16)
        Wlin = np.asarray(ins["Wlin"]).astype(np.float16)
        b1 = np.asarray(ins["b1"]).astype(np.float32)
        b2 = np.asarray(ins["b2"]).astype(np.float32)
        blin = np.asarray(ins["blin"]).astype(np.float32)

        w1p = np.zeros((512, 128), np.float16)
        w1p[:500] = W1
        b1v = b1[:, None]
        b2v = np.zeros((128, 1), np.float32)
        b2v[:40, 0] = b2
        blr = np.tile(blin[None, :], (128, 1)).astype(np.float32)
        wlt = np.zeros((104, 40), np.float16)
        wlt[0:40] = Wlin.T[0:40]
        wlt[64:104] = Wlin.T[40:80]

        self.run1_maps = []
        for i in range(N_CORES):
            import ml_dtypes
            xs = np.zeros((SH, 512), ml_dtypes.float8_e4m3)
            lo, hi = i * SH, min((i + 1) * SH, N_REAL)
            if hi > lo:
                xs[:hi - lo, :500] = x[lo:hi].astype(ml_dtypes.float8_e4m3)
            self.run1_maps.append({"xT": np.ascontiguousarray(xs.T), "w1": w1p})
        self.consts2 = {"w2": W2, "b1v": b1v}
        self.consts3 = {"wlt": wlt, "b2v": b2v, "blr": blr}

    def run(self, time_it=False):
        v = self.v
        t0 = time.time()
        for i in range(N_CORES):
            self.p1[i].upload(self.run1_maps[i])
        outs1 = _parallel([self.p1[i].call for i in range(N_CORES)])
        h_shards = [self.p1[i].results(outs1[i])["h"] for i in range(N_CORES)]
        table1 = np.zeros((NPAD, 256), h_shards[0].dtype)
        table1[:, :128] = np.concatenate(h_shards, axis=0)
        if v:
            print(f"[run1] done {time.time()-t0:.1f}s", flush=True)

        t0 = time.time()
        for i in range(N_CORES):
            pl = self.plans[i]
            self.p2[i].upload({"t1": table1, "ia": pl["ia"], "sa": pl["s_arr"],
                               **self.consts2})
        outs2 = _parallel([self.p2[i].call for i in range(N_CORES)])
        h2_shards = [self.p2[i].results(outs2[i])["h2"] for i in range(N_CORES)]
        table2 = np.zeros((NPAD, 256), h2_shards[0].dtype)
        table2[:, :40] = np.concatenate(h2_shards, axis=0)
        if v:
            print(f"[run2] done {time.time()-t0:.1f}s", flush=True)

        t0 = time.time()
        for i in range(N_CORES):
            pl = self.plans[i]
            self.p3[i].upload({"t2": table2, "ia": pl["ia"], "sa": pl["s_arr"],
                               **self.consts3})
        outs3 = _parallel([self.p3[i].call for i in range(N_CORES)])
        out_shards = [self.p3[i].results(outs3[i])["out"] for i in range(N_CORES)]
        result = np.concatenate(out_shards, axis=0)[:N_REAL]
        if v:
            print(f"[run3] done {time.time()-t0:.1f}s", flush=True)

        times = None
        if time_it:
            times = self.time_runs()
        return result, times

    def time_runs(self, reps=5):
        times = {}
        for name, progs in (("run1", self.p1), ("run2", self.p2), ("run3", self.p3)):
            best = float("inf")
            for _ in range(reps):
                barrier = threading.Barrier(N_CORES + 1)
                done = []

                def worker(p):
                    barrier.wait()
                    o = p.call()
                    jax.block_until_ready(o)
                    done.append(o)

                ts = [threading.Thread(target=worker, args=(p,)) for p in progs]
                for t in ts:
                    t.start()
                barrier.wait()
                t0 = time.time()
                for t in ts:
                    t.join()
                best = min(best, time.time() - t0)
            times[name] = best
        return times


_PIPELINE_CACHE = {}


def kernel(**inputs):
    key = "singleton"
    pl = _PIPELINE_CACHE.get(key)
    if pl is None or pl.graph_key != _graph_key(inputs):
        pl = Pipeline(inputs, verbose=False)
        pl.graph_key = _graph_key(inputs)
        _PIPELINE_CACHE[key] = pl
    else:
        pl.inputs = inputs
        pl._prepare_inputs()
    out, _ = pl.run(time_it=False)
    return out.astype(np.float32)


def _graph_key(inputs):
    ei = np.asarray(inputs["edge_index"])
    ek = np.asarray(inputs["edge_index_knn"])
    return (ei.shape, ek.shape, int(ei[:, 0].sum()), int(ei[:, -1].sum()),
            int(ek[:, 0].sum()), int(ek[:, -1].sum()))
